# revision 3
# baseline (speedup 1.0000x reference)
"""Causal self-attention (B=4, T=2048, E=1024, H=16, rope) on 8 trn2 NeuronCores.

Sharding: core c = 2*b + g handles batch b = c//2, head-group g = c%2
(8 of the 16 heads).  Each core:
  - projects its batch's x into q,k (feature-major, rope'd on chip) and v
    for its 8 heads,
  - runs causal attention entirely on-chip (S^T tiles as stationary
    operands, ones-augmented v gives softmax denominators for free),
  - AllGathers the fp16 attention output within the (g=0,g=1) pair and
    applies the full (row-complete) output projection, so no post-proj
    reduce is needed.
Host assembles out[b] from the pair's identical projected outputs.
"""
import sys

for _p in ("/opt/trn_rl_repo", "/root/.axon_site/_ro/trn_rl_repo"):
    if _p not in sys.path:
        sys.path.append(_p)

import numpy as np
from contextlib import ExitStack

import concourse.bass as bass
import concourse.tile as tile
from concourse import bacc, mybir
from concourse.bass_utils import run_bass_kernel_spmd

B, T, E = 4, 2048, 1024
H_TOT, D = 16, 64
HL = 8            # heads per core
F = HL * D        # 512 local q/k/v features
KB = E // 128     # 8 contraction blocks for qkv
TC = T // 512     # 4 time chunks (512 cols)
TT = T // 128     # 16 time tiles
ROPE_THETA = 10000.0
NEG = -1e30

f32 = mybir.dt.float32
f32r = mybir.dt.float32r
f16 = mybir.dt.float16


def _r(ap):
    return ap.bitcast(f32r)


def build_nc():
    nc = bacc.Bacc(None, target_bir_lowering=False, debug=False)

    xT = nc.declare_dram_parameter("xT", [E, T], f32, isOutput=False)
    wq = nc.declare_dram_parameter("wq", [E, F], f32, isOutput=False)
    wk = nc.declare_dram_parameter("wk", [E, F], f32, isOutput=False)
    wv = nc.declare_dram_parameter("wv", [E, F], f32, isOutput=False)
    bq = nc.declare_dram_parameter("bq", [1, F], f32, isOutput=False)
    bk = nc.declare_dram_parameter("bk", [1, F], f32, isOutput=False)
    bv = nc.declare_dram_parameter("bv", [1, F], f32, isOutput=False)
    wproj = nc.declare_dram_parameter("wproj", [E, E], f16, isOutput=False)
    bproj = nc.declare_dram_parameter("bproj", [1, E], f16, isOutput=False)
    ctab_d = nc.declare_dram_parameter("ctab", [128, T], f32, isOutput=False)
    stab_d = nc.declare_dram_parameter("stab", [128, T], f32, isOutput=False)
    perm_d = nc.declare_dram_parameter("perm", [128, 128], f32, isOutput=False)
    tri_d = nc.declare_dram_parameter("tri", [128, 128], f32, isOutput=False)
    ones_d = nc.declare_dram_parameter("ones", [1, 512], f32, isOutput=False)
    out_ext = nc.declare_dram_parameter("out", [T, E], f32, isOutput=True)

    ag_in = nc.dram_tensor("ag_in", [F, T], f16)
    ag_out = nc.dram_tensor("ag_out", [2, F, T], f16)

    with ExitStack() as ctx:
        tc = ctx.enter_context(tile.TileContext(nc))
        sres = ctx.enter_context(tc.tile_pool(name="res", bufs=1))
        swts = ctx.enter_context(tc.tile_pool(name="wts", bufs=3))
        stab = ctx.enter_context(tc.tile_pool(name="tab", bufs=2))
        sx = ctx.enter_context(tc.tile_pool(name="x", bufs=2))
        stmp = ctx.enter_context(tc.tile_pool(name="tmp", bufs=2))
        sp = ctx.enter_context(tc.tile_pool(name="p", bufs=3))
        sof = ctx.enter_context(tc.tile_pool(name="of", bufs=2))
        sout = ctx.enter_context(tc.tile_pool(name="out", bufs=2))
        ssm = ctx.enter_context(tc.tile_pool(name="sm", bufs=2))
        pps = ctx.enter_context(tc.tile_pool(name="ps", bufs=4, space="PSUM"))
        pac = ctx.enter_context(tc.tile_pool(name="ac", bufs=2, space="PSUM"))

        # ---- resident tiles
        qT_t = sres.tile([128, 4, T], f16, tag="qT")       # rope'd q, feature-major
        kT_t = sres.tile([128, 4, T], f16, tag="kT")
        v_t = sres.tile([128, TT, HL, D + 1], f16, tag="v")  # natural v + ones col
        ot_t = sres.tile([128, 4, T], f16, tag="ot")       # attention out, feature-major

        ones32 = sres.tile([1, 512], f32r, tag="ones32")
        ones16 = sres.tile([1, 128], f16, tag="ones16")
        nc.sync.dma_start(out=ones32, in_=ones_d[:, :].bitcast(f32r))
        nc.vector.memset(ones16, 1.0)
        nc.vector.memset(v_t[:, :, :, D:D + 1], 1.0)

        perm_t = sres.tile([128, 128], f32r, tag="perm")
        tri_t = sres.tile([128, 128], f32, tag="tri")
        nc.sync.dma_start(out=perm_t, in_=perm_d[:, :].bitcast(f32r))
        nc.sync.dma_start(out=tri_t, in_=tri_d[:, :])

        bq_t = sres.tile([1, F], f32r, tag="bq")
        bk_t = sres.tile([1, F], f32r, tag="bk")
        bv_t = sres.tile([1, F], f32r, tag="bv")
        bp_t = sres.tile([1, E], f16, tag="bp")
        nc.sync.dma_start(out=bq_t, in_=bq[:, :].bitcast(f32r))
        nc.sync.dma_start(out=bk_t, in_=bk[:, :].bitcast(f32r))
        nc.sync.dma_start(out=bv_t, in_=bv[:, :].bitcast(f32r))
        nc.sync.dma_start(out=bp_t, in_=bproj[:, :])

        ctab_t = stab.tile([128, T], f32, tag="tab")
        stab_t = stab.tile([128, T], f32, tag="tab")
        nc.sync.dma_start(out=ctab_t, in_=ctab_d[:, :])
        nc.sync.dma_start(out=stab_t, in_=stab_d[:, :])

        wq_t = swts.tile([128, KB, F], f32r, tag="w")
        wk_t = swts.tile([128, KB, F], f32r, tag="w")
        wv_t = swts.tile([128, KB, F], f32r, tag="w")
        xT_r = xT.rearrange("(a p) t -> p a t", p=128)
        nc.sync.dma_start(out=wq_t, in_=wq.rearrange("(a p) f -> p a f", p=128).bitcast(f32r))
        nc.sync.dma_start(out=wk_t, in_=wk.rearrange("(a p) f -> p a f", p=128).bitcast(f32r))
        nc.sync.dma_start(out=wv_t, in_=wv.rearrange("(a p) f -> p a f", p=128).bitcast(f32r))

        # ================= phase 1: qkv + rope =================
        for tcx in range(TC):
            x_t = sx.tile([128, KB, 512], f32r, tag="x")
            nc.sync.dma_start(out=x_t, in_=xT_r[:, :, tcx * 512:(tcx + 1) * 512].bitcast(f32r))

            for w_t, b_t, dst in ((wq_t, bq_t, qT_t), (wk_t, bk_t, kT_t)):
                for f in range(4):
                    ps_q = pps.tile([128, 512], f32, tag="mm")
                    for kb in range(KB):
                        nc.tensor.matmul(
                            ps_q[:, :],
                            w_t[:, kb, f * 128:(f + 1) * 128],
                            x_t[:, kb, :],
                            start=(kb == 0), stop=False,
                        )
                    nc.tensor.matmul(
                        ps_q[:, :],
                        b_t[0:1, f * 128:(f + 1) * 128],
                        ones32[0:1, :],
                        start=False, stop=True,
                    )
                    q32 = stmp.tile([128, 512], f32r, tag="t0")
                    nc.scalar.copy(q32[:, :], ps_q[:, :])
                    ps_p = pps.tile([128, 512], f32, tag="mm")
                    nc.tensor.matmul(ps_p[:, :], perm_t[:, :], q32[:, :],
                                     start=True, stop=True)
                    t1 = stmp.tile([128, 512], f32, tag="t1")
                    cs = slice(tcx * 512, (tcx + 1) * 512)
                    nc.vector.tensor_mul(t1[:, :], q32[:, :].bitcast(f32), ctab_t[:, cs])
                    t2 = stmp.tile([128, 512], f32, tag="t2")
                    nc.vector.tensor_mul(t2[:, :], ps_p[:, :], stab_t[:, cs])
                    nc.vector.tensor_add(dst[:, f, cs], t1[:, :], t2[:, :])

            for tl in range(4):
                tt = tcx * 4 + tl
                ps_v = pps.tile([128, 512], f32, tag="mm")
                for kb in range(KB):
                    nc.tensor.matmul(
                        ps_v[:, :],
                        x_t[:, kb, tl * 128:(tl + 1) * 128],
                        wv_t[:, kb, :],
                        start=(kb == 0), stop=False,
                    )
                nc.tensor.matmul(ps_v[:, :], ones32[0:1, 0:128], bv_t[0:1, :],
                                 start=False, stop=True)
                nc.scalar.copy(
                    v_t[:, tt, :, 0:D],
                    ps_v[:, :].rearrange("p (h d) -> p h d", h=HL),
                )

        # ================= phase 2: attention =================
        for h in range(HL):
            bp = (h % 2) * 64
            fi = h // 2
            for qc in range(TC):
                nkt = 4 * qc + 4
                ps_o = pac.tile([D + 1, 512], f32, tag="acc")
                qs = slice(qc * 512, (qc + 1) * 512)
                for kt in range(nkt):
                    j = kt - 4 * qc
                    w0 = max(j, 0) * 128          # first valid q col in this chunk
                    ps_s = pps.tile([128, 512], f32, tag="mm")
                    nc.tensor.matmul(
                        ps_s[:, w0:512],
                        kT_t[bp:bp + 64, fi, kt * 128:(kt + 1) * 128],
                        qT_t[bp:bp + 64, fi, qc * 512 + w0:(qc + 1) * 512],
                        start=True, stop=True,
                    )
                    if j >= 0:
                        nc.vector.tensor_add(
                            ps_s[:, w0:w0 + 128], ps_s[:, w0:w0 + 128], tri_t[:, :]
                        )
                    p_t = sp.tile([128, 512], f16, tag="p")
                    nc.scalar.activation(
                        p_t[:, w0:512], ps_s[:, w0:512],
                        mybir.ActivationFunctionType.Exp, scale=float(D) ** -0.5,
                    )
                    nc.tensor.matmul(
                        ps_o[:, w0:512],
                        v_t[:, kt, h, :],
                        p_t[:, w0:512],
                        start=(kt == 0), stop=(kt == nkt - 1),
                    )
                recip = ssm.tile([1, 512], f32, tag="rc")
                nc.vector.reciprocal(recip[:, :], ps_o[D:D + 1, :])
                bc = ssm.tile([64, 512], f32, tag="bc")
                nc.gpsimd.partition_broadcast(bc[:, :], recip[:, :])
                nc.vector.tensor_mul(ot_t[bp:bp + 64, fi, qs], ps_o[0:D, :], bc[:, :])

        # ================= phase 2.5: pair AllGather of attention out ====
        nc.sync.dma_start(out=ag_in.rearrange("(a p) t -> p a t", p=128), in_=ot_t)
        nc.gpsimd.collective_compute(
            "AllGather",
            mybir.AluOpType.bypass,
            ins=[ag_in[:, :]],
            outs=[ag_out[:, :, :]],
            replica_groups=[[0, 1], [2, 3], [4, 5], [6, 7]],
        )

        # ================= phase 3: output projection =================
        wp_t = swts.tile([128, KB, E], f16, tag="w")
        nc.sync.dma_start(out=wp_t, in_=wproj.rearrange("(a p) e -> p a e", p=128))
        ag_r = ag_out.rearrange("s (a p) t -> p (s a) t", p=128)

        for tt in range(TT):
            of_t = sof.tile([128, KB, 128], f16, tag="of")
            nc.sync.dma_start(out=of_t, in_=ag_r[:, :, tt * 128:(tt + 1) * 128])
            o_st = sout.tile([128, E], f32, tag="o")
            for nh in range(2):
                ps_pj = pac.tile([128, 512], f32, tag="acc")
                ns = slice(nh * 512, (nh + 1) * 512)
                for kb in range(KB):
                    nc.tensor.matmul(
                        ps_pj[:, :],
                        of_t[:, kb, :],
                        wp_t[:, kb, ns],
                        start=(kb == 0), stop=False,
                    )
                nc.tensor.matmul(ps_pj[:, :], ones16[0:1, :], bp_t[0:1, ns],
                                 start=False, stop=True)
                nc.vector.tensor_copy(o_st[:, ns], ps_pj[:, :])
            nc.sync.dma_start(out=out_ext[tt * 128:(tt + 1) * 128, :], in_=o_st)

    nc.compile()
    return nc


_NC = None


def _get_nc():
    global _NC
    if _NC is None:
        _NC = build_nc()
    return _NC


def _host_prep(x, Wqkv, bqkv, Wproj, bproj):
    """Build the 8 per-core input maps."""
    x = np.asarray(x, np.float32)
    Wqkv = np.asarray(Wqkv, np.float32)
    bqkv = np.asarray(bqkv, np.float32)
    Wproj = np.asarray(Wproj, np.float32)
    bproj = np.asarray(bproj, np.float32)

    perm_d = np.concatenate([np.arange(0, D, 2), np.arange(1, D, 2)])  # evens, odds

    # rope tables (feature-major; rows r: freq r%32, sign -/+ per 32-block)
    inv_freq = 1.0 / ROPE_THETA ** (np.arange(0, D, 2, dtype=np.float32) / D)
    freqs = np.arange(T, dtype=np.float32)[:, None] * inv_freq[None, :]  # (T, 32)
    cosf = np.cos(freqs).T.astype(np.float32)  # (32, T)
    sinf = np.sin(freqs).T.astype(np.float32)
    ctab = np.tile(cosf, (4, 1))                                   # (128, T)
    stab = np.concatenate([-sinf, sinf, -sinf, sinf], 0).astype(np.float32)

    # block-swap permutation matrix: out row m <- in row pi(m)
    pmat = np.zeros((128, 128), np.float32)
    for m in range(128):
        base = (m // 64) * 64
        r = m % 64
        pmat[base + (r + 32) % 64, m] = 1.0

    tri = np.where(
        np.arange(128)[:, None] <= np.arange(128)[None, :], 0.0, NEG
    ).astype(np.float32)

    maps = []
    for c in range(8):
        b, g = c // 2, c % 2
        heads = np.arange(8 * g, 8 * g + 8)
        # permuted q/k columns, natural v columns
        qcols = (heads[:, None] * D + perm_d[None, :]).ravel()
        vcols = (heads[:, None] * D + np.arange(D)[None, :]).ravel()
        maps.append({
            "xT": np.ascontiguousarray(x[b].T),
            "wq": np.ascontiguousarray(Wqkv[:, qcols]),
            "wk": np.ascontiguousarray(Wqkv[:, E + qcols]),
            "wv": np.ascontiguousarray(Wqkv[:, 2 * E + vcols]),
            "bq": np.ascontiguousarray(bqkv[qcols])[None, :],
            "bk": np.ascontiguousarray(bqkv[E + qcols])[None, :],
            "bv": np.ascontiguousarray(bqkv[2 * E + vcols])[None, :],
            "wproj": np.ascontiguousarray(Wproj.astype(np.float16)),
            "bproj": np.ascontiguousarray(bproj.astype(np.float16))[None, :],
            "ctab": ctab,
            "stab": stab,
            "perm": pmat,
            "tri": tri,
            "ones": np.ones((1, 512), np.float32),
        })
    return maps


def kernel(x, Wqkv, bqkv, Wproj, bproj):
    nc = _get_nc()
    in_maps = _host_prep(x, Wqkv, bqkv, Wproj, bproj)
    res = run_bass_kernel_spmd(nc, in_maps, list(range(8)))
    out = np.empty((B, T, E), np.float32)
    for b in range(B):
        out[b, :T // 2] = res.results[2 * b]["out"][:T // 2]
        out[b, T // 2:] = res.results[2 * b + 1]["out"][T // 2:]
    return out


if __name__ == "__main__":
    rng = np.random.default_rng(0)
    x = rng.standard_normal((B, T, E), dtype=np.float32)
    Wqkv = rng.standard_normal((E, 3 * E), dtype=np.float32) * 0.02
    bqkv = rng.standard_normal((3 * E,), dtype=np.float32) * 0.02
    Wproj = rng.standard_normal((E, E), dtype=np.float32) * 0.02
    bproj = rng.standard_normal((E,), dtype=np.float32) * 0.02
    o = kernel(x=x, Wqkv=Wqkv, bqkv=bqkv, Wproj=Wproj, bproj=bproj)
    print("out", o.shape, o.dtype, float(np.abs(o).max()))


# revision 5
# speedup vs baseline: 1.1649x; 1.1649x over previous
"""Causal self-attention (B=4, T=2048, E=1024, H=16, rope) on 8 trn2 NeuronCores.

Sharding: core c = 2*b + g handles batch b = c//2, head-group g = c%2
(8 of the 16 heads).  Each core:
  - projects its batch's x into q,k (feature-major, rope'd on chip) and v
    for its 8 heads (fp16 matmuls, fp32 accumulate),
  - runs causal attention entirely on-chip (S^T tiles as stationary
    operands, ones-augmented v gives softmax denominators for free),
  - AllGathers the fp16 attention output within the (g=0,g=1) pair and
    applies the full (row-complete) output projection, so no post-proj
    reduce is needed.
Host assembles out[b] from the pair's identical projected outputs.
"""
import sys

for _p in ("/opt/trn_rl_repo", "/root/.axon_site/_ro/trn_rl_repo"):
    if _p not in sys.path:
        sys.path.append(_p)

import numpy as np
from contextlib import ExitStack

import concourse.bass as bass
import concourse.tile as tile
from concourse import bacc, mybir
from concourse.bass_utils import run_bass_kernel_spmd

B, T, E = 4, 2048, 1024
H_TOT, D = 16, 64
HL = 8            # heads per core
F = HL * D        # 512 local q/k/v features
KB = E // 128     # 8 contraction blocks for qkv
TC = T // 512     # 4 time chunks (512 cols)
TT = T // 128     # 16 time tiles
ROPE_THETA = 10000.0

f32 = mybir.dt.float32
f16 = mybir.dt.float16


def build_nc():
    nc = bacc.Bacc(None, target_bir_lowering=False, debug=False)

    xT = nc.declare_dram_parameter("xT", [E, T], f16, isOutput=False)
    wq = nc.declare_dram_parameter("wq", [E, F], f16, isOutput=False)
    wk = nc.declare_dram_parameter("wk", [E, F], f16, isOutput=False)
    wv = nc.declare_dram_parameter("wv", [E, F], f16, isOutput=False)
    bq = nc.declare_dram_parameter("bq", [128, 4], f32, isOutput=False)  # feature-major cols
    bk = nc.declare_dram_parameter("bk", [128, 4], f32, isOutput=False)
    bv = nc.declare_dram_parameter("bv", [1, F], f16, isOutput=False)
    wproj = nc.declare_dram_parameter("wproj", [E, E], f16, isOutput=False)
    bproj = nc.declare_dram_parameter("bproj", [1, E], f16, isOutput=False)
    ctab_d = nc.declare_dram_parameter("ctab", [128, T], f16, isOutput=False)
    stab_d = nc.declare_dram_parameter("stab", [128, T], f16, isOutput=False)
    perm_d = nc.declare_dram_parameter("perm", [128, 128], f16, isOutput=False)
    tri_d = nc.declare_dram_parameter("tri", [128, 128], f16, isOutput=False)  # 0/1 mult mask
    ones_d = nc.declare_dram_parameter("ones", [1, 512], f16, isOutput=False)
    out_ext = nc.declare_dram_parameter("out", [T, E], f32, isOutput=True)

    ag_in = nc.dram_tensor("ag_in", [F, T], f16)
    ag_out = nc.dram_tensor("ag_out", [2, F, T], f16)

    with ExitStack() as ctx:
        tc = ctx.enter_context(tile.TileContext(nc))
        sres = ctx.enter_context(tc.tile_pool(name="res", bufs=1))
        swts = ctx.enter_context(tc.tile_pool(name="wts", bufs=3))
        stab = ctx.enter_context(tc.tile_pool(name="tab", bufs=2))
        sx = ctx.enter_context(tc.tile_pool(name="x", bufs=2))
        stmp = ctx.enter_context(tc.tile_pool(name="tmp", bufs=2))
        sp = ctx.enter_context(tc.tile_pool(name="p", bufs=3))
        sof = ctx.enter_context(tc.tile_pool(name="of", bufs=2))
        sout = ctx.enter_context(tc.tile_pool(name="out", bufs=2))
        ssm = ctx.enter_context(tc.tile_pool(name="sm", bufs=2))
        pps = ctx.enter_context(tc.tile_pool(name="ps", bufs=2, space="PSUM"))
        pac = ctx.enter_context(tc.tile_pool(name="ac", bufs=3, space="PSUM"))

        # ---- resident tiles
        qT_t = sres.tile([128, 4, T], f16, tag="qT")       # rope'd q, feature-major
        kT_t = sres.tile([128, 4, T], f16, tag="kT")
        v_t = sres.tile([128, TT, HL, D + 1], f16, tag="v")  # natural v + ones col
        ot_t = sres.tile([128, 4, T], f16, tag="ot")       # attention out, feature-major

        ones16 = sres.tile([1, 512], f16, tag="ones16")
        nc.sync.dma_start(out=ones16, in_=ones_d[:, :])
        nc.vector.memset(v_t[:, :, :, D:D + 1], 1.0)

        perm_t = sres.tile([128, 128], f16, tag="perm")
        tri_t = sres.tile([128, 128], f16, tag="tri")
        nc.sync.dma_start(out=perm_t, in_=perm_d[:, :])
        nc.sync.dma_start(out=tri_t, in_=tri_d[:, :])

        bq_t = sres.tile([128, 4], f32, tag="bq")
        bk_t = sres.tile([128, 4], f32, tag="bk")
        bv_t = sres.tile([1, F], f16, tag="bv")
        bp_t = sres.tile([1, E], f16, tag="bp")
        nc.sync.dma_start(out=bq_t, in_=bq[:, :])
        nc.sync.dma_start(out=bk_t, in_=bk[:, :])
        nc.sync.dma_start(out=bv_t, in_=bv[:, :])
        nc.sync.dma_start(out=bp_t, in_=bproj[:, :])

        ctab_t = stab.tile([128, T], f16, tag="tab")
        stab_t = stab.tile([128, T], f16, tag="tab")
        nc.sync.dma_start(out=ctab_t, in_=ctab_d[:, :])
        nc.sync.dma_start(out=stab_t, in_=stab_d[:, :])

        wq_t = swts.tile([128, KB, F], f16, tag="w3")
        wk_t = swts.tile([128, KB, F], f16, tag="w3")
        wv_t = swts.tile([128, KB, F], f16, tag="w3")
        xT_r = xT.rearrange("(a p) t -> p a t", p=128)
        nc.sync.dma_start(out=wq_t, in_=wq.rearrange("(a p) f -> p a f", p=128))
        nc.sync.dma_start(out=wk_t, in_=wk.rearrange("(a p) f -> p a f", p=128))
        nc.sync.dma_start(out=wv_t, in_=wv.rearrange("(a p) f -> p a f", p=128))

        # ================= phase 1: qkv + rope =================
        for tcx in range(TC):
            x_t = sx.tile([128, KB, 512], f16, tag="x")
            nc.sync.dma_start(out=x_t, in_=xT_r[:, :, tcx * 512:(tcx + 1) * 512])
            cs = slice(tcx * 512, (tcx + 1) * 512)

            for w_t, b_t, dst in ((wq_t, bq_t, qT_t), (wk_t, bk_t, kT_t)):
                for f in range(4):
                    ps2 = pps.tile([128, 1024], f32, tag="mm")
                    ps_q = ps2[:, 0:512]
                    ps_p = ps2[:, 512:1024]
                    for kb in range(KB):
                        nc.tensor.matmul(
                            ps_q,
                            w_t[:, kb, f * 128:(f + 1) * 128],
                            x_t[:, kb, :],
                            start=(kb == 0), stop=(kb == KB - 1),
                        )
                    # bias add (per-partition col) fused into psum->sbuf move
                    q16 = stmp.tile([128, 512], f16, tag="t0")
                    nc.vector.tensor_scalar_add(q16[:, :], ps_q, b_t[:, f:f + 1])
                    nc.tensor.matmul(ps_p, perm_t[:, :], q16[:, :],
                                     start=True, stop=True)
                    t1 = stmp.tile([128, 512], f16, tag="t1")
                    nc.vector.tensor_mul(t1[:, :], q16[:, :], ctab_t[:, cs])
                    t2 = stmp.tile([128, 512], f16, tag="t2")
                    nc.vector.tensor_mul(t2[:, :], ps_p, stab_t[:, cs])
                    nc.vector.tensor_add(dst[:, f, cs], t1[:, :], t2[:, :])

            for tl in range(4):
                tt = tcx * 4 + tl
                ps2 = pps.tile([128, 1024], f32, tag="mm")
                ps_v = ps2[:, 0:512]
                for kb in range(KB):
                    nc.tensor.matmul(
                        ps_v,
                        x_t[:, kb, tl * 128:(tl + 1) * 128],
                        wv_t[:, kb, :],
                        start=(kb == 0), stop=False,
                    )
                nc.tensor.matmul(ps_v, ones16[0:1, 0:128], bv_t[0:1, :],
                                 start=False, stop=True)
                nc.scalar.copy(
                    v_t[:, tt, :, 0:D],
                    ps_v.rearrange("p (h d) -> p h d", h=HL),
                )

        # ================= phase 2: attention =================
        for h in range(HL):
            bp = (h % 2) * 64
            fi = h // 2
            for qc in range(TC):
                nkt = 4 * qc + 4
                ps_o = pac.tile([D + 1, 512], f32, tag="acc")
                qs = slice(qc * 512, (qc + 1) * 512)
                for m in range(nkt // 2):
                    ps_s = pps.tile([128, 1024], f32, tag="mm")
                    p_t = sp.tile([128, 1024], f16, tag="p")
                    for half in range(2):
                        kt = 2 * m + half
                        j = kt - 4 * qc
                        w0 = max(j, 0) * 128
                        nc.tensor.matmul(
                            ps_s[:, half * 512 + w0:half * 512 + 512],
                            kT_t[bp:bp + 64, fi, kt * 128:(kt + 1) * 128],
                            qT_t[bp:bp + 64, fi, qc * 512 + w0:(qc + 1) * 512],
                            start=True, stop=True,
                        )
                    nc.scalar.activation(
                        p_t[:, :], ps_s[:, :],
                        mybir.ActivationFunctionType.Exp, scale=float(D) ** -0.5,
                    )
                    for half in range(2):
                        kt = 2 * m + half
                        j = kt - 4 * qc
                        w0 = max(j, 0) * 128
                        if j >= 0:
                            ms = slice(half * 512 + w0, half * 512 + w0 + 128)
                            nc.vector.tensor_mul(p_t[:, ms], p_t[:, ms], tri_t[:, :])
                        nc.tensor.matmul(
                            ps_o[:, w0:512],
                            v_t[:, kt, h, :],
                            p_t[:, half * 512 + w0:half * 512 + 512],
                            start=(kt == 0), stop=(kt == nkt - 1),
                        )
                recip = ssm.tile([1, 512], f32, tag="rc")
                nc.vector.reciprocal(recip[:, :], ps_o[D:D + 1, :])
                bc = ssm.tile([64, 512], f32, tag="bc")
                nc.gpsimd.partition_broadcast(bc[:, :], recip[:, :])
                nc.vector.tensor_mul(ot_t[bp:bp + 64, fi, qs], ps_o[0:D, :], bc[:, :])

        # ================= phase 2.5: pair AllGather of attention out ====
        nc.sync.dma_start(out=ag_in.rearrange("(a p) t -> p a t", p=128), in_=ot_t)
        nc.gpsimd.collective_compute(
            "AllGather",
            mybir.AluOpType.bypass,
            ins=[ag_in[:, :]],
            outs=[ag_out[:, :, :]],
            replica_groups=[[0, 1], [2, 3], [4, 5], [6, 7]],
        )

        # ================= phase 3: output projection =================
        wp_t = swts.tile([128, KB, E], f16, tag="wp")
        nc.sync.dma_start(out=wp_t, in_=wproj.rearrange("(a p) e -> p a e", p=128))
        ag_r = ag_out.rearrange("s (a p) t -> p (s a) t", p=128)

        for tt in range(TT):
            of_t = sof.tile([128, KB, 128], f16, tag="of")
            nc.sync.dma_start(out=of_t, in_=ag_r[:, :, tt * 128:(tt + 1) * 128])
            o_st = sout.tile([128, E], f32, tag="o")
            ps2 = pps.tile([128, 1024], f32, tag="mm")
            for nh in range(2):
                ps_pj = ps2[:, nh * 512:(nh + 1) * 512]
                ns = slice(nh * 512, (nh + 1) * 512)
                for kb in range(KB):
                    nc.tensor.matmul(
                        ps_pj,
                        of_t[:, kb, :],
                        wp_t[:, kb, ns],
                        start=(kb == 0), stop=False,
                    )
                nc.tensor.matmul(ps_pj, ones16[0:1, 0:128], bp_t[0:1, ns],
                                 start=False, stop=True)
            nc.vector.tensor_copy(o_st[:, :], ps2[:, :])
            nc.sync.dma_start(out=out_ext[tt * 128:(tt + 1) * 128, :], in_=o_st)

    nc.compile()
    return nc


_NC = None


def _get_nc():
    global _NC
    if _NC is None:
        _NC = build_nc()
    return _NC


def _host_prep(x, Wqkv, bqkv, Wproj, bproj):
    """Build the 8 per-core input maps."""
    x = np.asarray(x, np.float32)
    Wqkv = np.asarray(Wqkv, np.float32)
    bqkv = np.asarray(bqkv, np.float32)
    Wproj = np.asarray(Wproj, np.float32)
    bproj = np.asarray(bproj, np.float32)

    perm_d = np.concatenate([np.arange(0, D, 2), np.arange(1, D, 2)])  # evens, odds

    # rope tables (feature-major; rows r: freq r%32, sign -/+ per 32-block)
    inv_freq = 1.0 / ROPE_THETA ** (np.arange(0, D, 2, dtype=np.float32) / D)
    freqs = np.arange(T, dtype=np.float32)[:, None] * inv_freq[None, :]  # (T, 32)
    cosf = np.cos(freqs).T.astype(np.float32)  # (32, T)
    sinf = np.sin(freqs).T.astype(np.float32)
    ctab = np.tile(cosf, (4, 1)).astype(np.float16)                 # (128, T)
    stab = np.concatenate([-sinf, sinf, -sinf, sinf], 0).astype(np.float16)

    # block-swap permutation matrix: out row m <- in row pi(m)
    pmat = np.zeros((128, 128), np.float16)
    for m in range(128):
        base = (m // 64) * 64
        r = m % 64
        pmat[base + (r + 32) % 64, m] = 1.0

    tri = (np.arange(128)[:, None] <= np.arange(128)[None, :]).astype(np.float16)

    maps = []
    for c in range(8):
        b, g = c // 2, c % 2
        heads = np.arange(8 * g, 8 * g + 8)
        # permuted q/k columns, natural v columns
        qcols = (heads[:, None] * D + perm_d[None, :]).ravel()
        vcols = (heads[:, None] * D + np.arange(D)[None, :]).ravel()
        maps.append({
            "xT": np.ascontiguousarray(x[b].T.astype(np.float16)),
            "wq": np.ascontiguousarray(Wqkv[:, qcols].astype(np.float16)),
            "wk": np.ascontiguousarray(Wqkv[:, E + qcols].astype(np.float16)),
            "wv": np.ascontiguousarray(Wqkv[:, 2 * E + vcols].astype(np.float16)),
            "bq": np.ascontiguousarray(bqkv[qcols].reshape(4, 128).T.astype(np.float32)),
            "bk": np.ascontiguousarray(bqkv[E + qcols].reshape(4, 128).T.astype(np.float32)),
            "bv": np.ascontiguousarray(bqkv[2 * E + vcols].astype(np.float16))[None, :],
            "wproj": np.ascontiguousarray(Wproj.astype(np.float16)),
            "bproj": np.ascontiguousarray(bproj.astype(np.float16))[None, :],
            "ctab": ctab,
            "stab": stab,
            "perm": pmat,
            "tri": tri,
            "ones": np.ones((1, 512), np.float16),
        })
    return maps


def kernel(x, Wqkv, bqkv, Wproj, bproj):
    nc = _get_nc()
    in_maps = _host_prep(x, Wqkv, bqkv, Wproj, bproj)
    res = run_bass_kernel_spmd(nc, in_maps, list(range(8)))
    out = np.empty((B, T, E), np.float32)
    for b in range(B):
        out[b, :T // 2] = res.results[2 * b]["out"][:T // 2]
        out[b, T // 2:] = res.results[2 * b + 1]["out"][T // 2:]
    return out


if __name__ == "__main__":
    rng = np.random.default_rng(0)
    x = rng.standard_normal((B, T, E), dtype=np.float32)
    Wqkv = rng.standard_normal((E, 3 * E), dtype=np.float32) * 0.02
    bqkv = rng.standard_normal((3 * E,), dtype=np.float32) * 0.02
    Wproj = rng.standard_normal((E, E), dtype=np.float32) * 0.02
    bproj = rng.standard_normal((E,), dtype=np.float32) * 0.02
    o = kernel(x=x, Wqkv=Wqkv, bqkv=bqkv, Wproj=Wproj, bproj=bproj)
    print("out", o.shape, o.dtype, float(np.abs(o).max()))


# revision 9
# speedup vs baseline: 1.4406x; 1.2366x over previous
"""Causal self-attention (B=4, T=2048, E=1024, H=16, rope) on 8 trn2 NeuronCores.

Sharding: core c = 2*b + g handles batch b = c//2, head-group g = c%2
(8 of the 16 heads).  Each core:
  - projects its batch's x into q,k (feature-major, rope'd on chip) and v
    for its 8 heads (fp16 matmuls, fp32 accumulate),
  - runs causal attention entirely on-chip (S^T tiles as stationary
    operands, ones-augmented v gives softmax denominators for free),
  - AllGathers the fp16 attention output within the (g=0,g=1) pair and
    applies the full (row-complete) output projection, so no post-proj
    reduce is needed.
Host assembles out[b] from the pair's identical projected outputs.
"""
import sys

for _p in ("/opt/trn_rl_repo", "/root/.axon_site/_ro/trn_rl_repo"):
    if _p not in sys.path:
        sys.path.append(_p)

import numpy as np
from contextlib import ExitStack

import concourse.bass as bass
import concourse.tile as tile
from concourse import bacc, mybir
from concourse.bass_utils import run_bass_kernel_spmd

B, T, E = 4, 2048, 1024
H_TOT, D = 16, 64
HL = 8            # heads per core
F = HL * D        # 512 local q/k/v features
KB = E // 128     # 8 contraction blocks for qkv
TC = T // 512     # 4 time chunks (512 cols)
TT = T // 128     # 16 time tiles
ROPE_THETA = 10000.0

f32 = mybir.dt.float32
f16 = mybir.dt.float16


def build_nc():
    nc = bacc.Bacc(None, target_bir_lowering=False, debug=False)

    xT = nc.declare_dram_parameter("xT", [E, T], f16, isOutput=False)
    wq = nc.declare_dram_parameter("wq", [E, F], f16, isOutput=False)
    wk = nc.declare_dram_parameter("wk", [E, F], f16, isOutput=False)
    wv = nc.declare_dram_parameter("wv", [E, F], f16, isOutput=False)
    bq = nc.declare_dram_parameter("bq", [128, 4], f32, isOutput=False)  # feature-major cols
    bk = nc.declare_dram_parameter("bk", [128, 4], f32, isOutput=False)
    bv = nc.declare_dram_parameter("bv", [1, F], f16, isOutput=False)
    wproj = nc.declare_dram_parameter("wproj", [E, E], f16, isOutput=False)
    bproj = nc.declare_dram_parameter("bproj", [1, E], f16, isOutput=False)
    ctab_d = nc.declare_dram_parameter("ctab", [128, T], f16, isOutput=False)
    stab_d = nc.declare_dram_parameter("stab", [128, T], f16, isOutput=False)
    perm_d = nc.declare_dram_parameter("perm", [128, 128], f16, isOutput=False)
    tri_d = nc.declare_dram_parameter("tri", [128, 128], f16, isOutput=False)  # 0/1 mult mask
    ones_d = nc.declare_dram_parameter("ones", [1, 512], f16, isOutput=False)
    out_ext = nc.declare_dram_parameter("out", [T, E], f32, isOutput=True)

    ag_in = nc.dram_tensor("ag_in", [F, T], f16)
    ag_out = nc.dram_tensor("ag_out", [2, F, T], f16)

    with ExitStack() as ctx:
        tc = ctx.enter_context(tile.TileContext(nc))
        sres = ctx.enter_context(tc.tile_pool(name="res", bufs=1))
        swts = ctx.enter_context(tc.tile_pool(name="wts", bufs=3))
        stab = ctx.enter_context(tc.tile_pool(name="tab", bufs=2))
        sx = ctx.enter_context(tc.tile_pool(name="x", bufs=2))
        stmp = ctx.enter_context(tc.tile_pool(name="tmp", bufs=2))
        sp = ctx.enter_context(tc.tile_pool(name="p", bufs=3))
        sof = ctx.enter_context(tc.tile_pool(name="of", bufs=2))
        sout = ctx.enter_context(tc.tile_pool(name="out", bufs=2))
        ssm = ctx.enter_context(tc.tile_pool(name="sm", bufs=2))
        pps = ctx.enter_context(tc.tile_pool(name="ps", bufs=3, space="PSUM"))
        pac = ctx.enter_context(tc.tile_pool(name="ac", bufs=2, space="PSUM"))

        # ---- resident tiles
        qT_t = sres.tile([128, 4, T], f16, tag="qT")       # rope'd q, feature-major
        kT_t = sres.tile([128, 4, T], f16, tag="kT")
        v_t = sres.tile([128, TT, HL, D + 1], f16, tag="v")  # natural v + ones col
        ot_t = sres.tile([128, 4, T], f16, tag="ot")       # attention out, feature-major

        ones16 = sres.tile([1, 512], f16, tag="ones16")
        nc.sync.dma_start(out=ones16, in_=ones_d[:, :])
        nc.vector.memset(v_t[:, :, :, D:D + 1], 1.0)

        perm_t = sres.tile([128, 128], f16, tag="perm")
        tri_t = sres.tile([128, 128], f16, tag="tri")
        nc.sync.dma_start(out=perm_t, in_=perm_d[:, :])
        nc.sync.dma_start(out=tri_t, in_=tri_d[:, :])

        bq_t = sres.tile([128, 4], f32, tag="bq")
        bk_t = sres.tile([128, 4], f32, tag="bk")
        bv_t = sres.tile([1, F], f16, tag="bv")
        bp_t = sres.tile([1, E], f16, tag="bp")
        nc.sync.dma_start(out=bq_t, in_=bq[:, :])
        nc.sync.dma_start(out=bk_t, in_=bk[:, :])
        nc.sync.dma_start(out=bv_t, in_=bv[:, :])
        nc.sync.dma_start(out=bp_t, in_=bproj[:, :])

        ctab_t = stab.tile([128, T], f16, tag="tab")
        stab_t = stab.tile([128, T], f16, tag="tab")
        nc.sync.dma_start(out=ctab_t, in_=ctab_d[:, :])
        nc.sync.dma_start(out=stab_t, in_=stab_d[:, :])

        wq_t = swts.tile([128, KB, F], f16, tag="w3")
        wk_t = swts.tile([128, KB, F], f16, tag="w3")
        wv_t = swts.tile([128, KB, F], f16, tag="w3")
        xT_r = xT.rearrange("(a p) t -> p a t", p=128)
        nc.sync.dma_start(out=wq_t, in_=wq.rearrange("(a p) f -> p a f", p=128))
        nc.sync.dma_start(out=wk_t, in_=wk.rearrange("(a p) f -> p a f", p=128))
        nc.sync.dma_start(out=wv_t, in_=wv.rearrange("(a p) f -> p a f", p=128))

        # ============ interleaved qkv-chunk + attention group per qc ======
        # attention chains (h, qc) only need q/k/v for time < (qc+1)*512,
        # so producing chunk qc then running all 8 heads' chains for qc keeps
        # the PE dense (qkv matmuls fill the gaps the exp pipeline leaves).
        def qkv_chunk(tcx):
            x_t = sx.tile([128, KB, 512], f16, tag="x")
            nc.sync.dma_start(out=x_t, in_=xT_r[:, :, tcx * 512:(tcx + 1) * 512])
            cs = slice(tcx * 512, (tcx + 1) * 512)

            for w_t, b_t, dst in ((wq_t, bq_t, qT_t), (wk_t, bk_t, kT_t)):
                for f in range(4):
                    ps2 = pps.tile([128, 1024], f32, tag="mm")
                    ps_q = ps2[:, 0:512]
                    ps_p = ps2[:, 512:1024]
                    for kb in range(KB):
                        nc.tensor.matmul(
                            ps_q,
                            w_t[:, kb, f * 128:(f + 1) * 128],
                            x_t[:, kb, :],
                            start=(kb == 0), stop=(kb == KB - 1),
                        )
                    # bias add (per-partition col) fused into psum->sbuf move
                    q16 = stmp.tile([128, 512], f16, tag="t0")
                    nc.vector.tensor_scalar_add(q16[:, :], ps_q, b_t[:, f:f + 1])
                    nc.tensor.matmul(ps_p, perm_t[:, :], q16[:, :],
                                     start=True, stop=True)
                    t1 = stmp.tile([128, 512], f16, tag="t1")
                    nc.vector.tensor_mul(t1[:, :], q16[:, :], ctab_t[:, cs])
                    t2 = stmp.tile([128, 512], f16, tag="t2")
                    nc.vector.tensor_mul(t2[:, :], ps_p, stab_t[:, cs])
                    nc.vector.tensor_add(dst[:, f, cs], t1[:, :], t2[:, :])

            for tl in range(4):
                tt = tcx * 4 + tl
                ps2 = pps.tile([128, 1024], f32, tag="mm")
                ps_v = ps2[:, 0:512]
                for kb in range(KB):
                    nc.tensor.matmul(
                        ps_v,
                        x_t[:, kb, tl * 128:(tl + 1) * 128],
                        wv_t[:, kb, :],
                        start=(kb == 0), stop=False,
                    )
                nc.tensor.matmul(ps_v, ones16[0:1, 0:128], bv_t[0:1, :],
                                 start=False, stop=True)
                nc.scalar.copy(
                    v_t[:, tt, :, 0:D],
                    ps_v.rearrange("p (h d) -> p h d", h=HL),
                )

        for qc in range(TC):
            qkv_chunk(qc)
            qs = slice(qc * 512, (qc + 1) * 512)
            for h in range(HL):
                bp = (h % 2) * 64
                fi = h // 2
                nkt = 4 * qc + 4
                ps_o = pac.tile([D + 1, 512], f32, tag="acc")
                for m in range(nkt // 2):
                    ps_s = pps.tile([128, 1024], f32, tag="mm")
                    p_t = sp.tile([128, 1024], f16, tag="p")
                    for half in range(2):
                        kt = 2 * m + half
                        j = kt - 4 * qc
                        w0 = max(j, 0) * 128
                        nc.tensor.matmul(
                            ps_s[:, half * 512 + w0:half * 512 + 512],
                            kT_t[bp:bp + 64, fi, kt * 128:(kt + 1) * 128],
                            qT_t[bp:bp + 64, fi, qc * 512 + w0:(qc + 1) * 512],
                            start=True, stop=True,
                        )
                    nc.scalar.activation(
                        p_t[:, :], ps_s[:, :],
                        mybir.ActivationFunctionType.Exp, scale=float(D) ** -0.5,
                    )
                    for half in range(2):
                        kt = 2 * m + half
                        j = kt - 4 * qc
                        w0 = max(j, 0) * 128
                        if j >= 0:
                            ms = slice(half * 512 + w0, half * 512 + w0 + 128)
                            nc.vector.tensor_mul(p_t[:, ms], p_t[:, ms], tri_t[:, :])
                        nc.tensor.matmul(
                            ps_o[:, w0:512],
                            v_t[:, kt, h, :],
                            p_t[:, half * 512 + w0:half * 512 + 512],
                            start=(kt == 0), stop=(kt == nkt - 1),
                        )
                # normalize: denominators -> sbuf -> fast reciprocal -> bcast
                sums_sb = ssm.tile([1, 512], f32, tag="sums")
                nc.scalar.copy(sums_sb[:, :], ps_o[D:D + 1, :])
                recip = ssm.tile([1, 512], f32, tag="rc")
                nc.vector.reciprocal_approx_fast(out=recip[:, :], in_=sums_sb[:, :])
                bc = ssm.tile([64, 512], f32, tag="bc")
                nc.gpsimd.partition_broadcast(bc[:, :], recip[:, :])
                nc.vector.tensor_mul(ot_t[bp:bp + 64, fi, qs], ps_o[0:D, :], bc[:, :])

        # ================= phase 2.5: pair AllGather of attention out ====
        nc.sync.dma_start(out=ag_in.rearrange("(a p) t -> p a t", p=128), in_=ot_t)
        nc.gpsimd.collective_compute(
            "AllGather",
            mybir.AluOpType.bypass,
            ins=[ag_in[:, :]],
            outs=[ag_out[:, :, :]],
            replica_groups=[[0, 1], [2, 3], [4, 5], [6, 7]],
        )

        # ================= phase 3: output projection =================
        wp_t = swts.tile([128, KB, E], f16, tag="wp")
        nc.sync.dma_start(out=wp_t, in_=wproj.rearrange("(a p) e -> p a e", p=128))
        ag_r = ag_out.rearrange("s (a p) t -> p (s a) t", p=128)

        for tt in range(TT):
            of_t = sof.tile([128, KB, 128], f16, tag="of")
            nc.sync.dma_start(out=of_t, in_=ag_r[:, :, tt * 128:(tt + 1) * 128])
            o_st = sout.tile([128, E], f32, tag="o")
            ps2 = pps.tile([128, 1024], f32, tag="mm")
            for nh in range(2):
                ps_pj = ps2[:, nh * 512:(nh + 1) * 512]
                ns = slice(nh * 512, (nh + 1) * 512)
                for kb in range(KB):
                    nc.tensor.matmul(
                        ps_pj,
                        of_t[:, kb, :],
                        wp_t[:, kb, ns],
                        start=(kb == 0), stop=False,
                    )
                nc.tensor.matmul(ps_pj, ones16[0:1, 0:128], bp_t[0:1, ns],
                                 start=False, stop=True)
            nc.vector.tensor_copy(o_st[:, :], ps2[:, :])
            nc.sync.dma_start(out=out_ext[tt * 128:(tt + 1) * 128, :], in_=o_st)

    nc.compile()
    return nc


_NC = None


def _get_nc():
    global _NC
    if _NC is None:
        _NC = build_nc()
    return _NC


def _host_prep(x, Wqkv, bqkv, Wproj, bproj):
    """Build the 8 per-core input maps."""
    x = np.asarray(x, np.float32)
    Wqkv = np.asarray(Wqkv, np.float32)
    bqkv = np.asarray(bqkv, np.float32)
    Wproj = np.asarray(Wproj, np.float32)
    bproj = np.asarray(bproj, np.float32)

    perm_d = np.concatenate([np.arange(0, D, 2), np.arange(1, D, 2)])  # evens, odds

    # rope tables (feature-major; rows r: freq r%32, sign -/+ per 32-block)
    inv_freq = 1.0 / ROPE_THETA ** (np.arange(0, D, 2, dtype=np.float32) / D)
    freqs = np.arange(T, dtype=np.float32)[:, None] * inv_freq[None, :]  # (T, 32)
    cosf = np.cos(freqs).T.astype(np.float32)  # (32, T)
    sinf = np.sin(freqs).T.astype(np.float32)
    ctab = np.tile(cosf, (4, 1)).astype(np.float16)                 # (128, T)
    stab = np.concatenate([-sinf, sinf, -sinf, sinf], 0).astype(np.float16)

    # block-swap permutation matrix: out row m <- in row pi(m)
    pmat = np.zeros((128, 128), np.float16)
    for m in range(128):
        base = (m // 64) * 64
        r = m % 64
        pmat[base + (r + 32) % 64, m] = 1.0

    tri = (np.arange(128)[:, None] <= np.arange(128)[None, :]).astype(np.float16)

    maps = []
    for c in range(8):
        b, g = c // 2, c % 2
        heads = np.arange(8 * g, 8 * g + 8)
        # permuted q/k columns, natural v columns
        qcols = (heads[:, None] * D + perm_d[None, :]).ravel()
        vcols = (heads[:, None] * D + np.arange(D)[None, :]).ravel()
        maps.append({
            "xT": np.ascontiguousarray(x[b].T.astype(np.float16)),
            "wq": np.ascontiguousarray(Wqkv[:, qcols].astype(np.float16)),
            "wk": np.ascontiguousarray(Wqkv[:, E + qcols].astype(np.float16)),
            "wv": np.ascontiguousarray(Wqkv[:, 2 * E + vcols].astype(np.float16)),
            "bq": np.ascontiguousarray(bqkv[qcols].reshape(4, 128).T.astype(np.float32)),
            "bk": np.ascontiguousarray(bqkv[E + qcols].reshape(4, 128).T.astype(np.float32)),
            "bv": np.ascontiguousarray(bqkv[2 * E + vcols].astype(np.float16))[None, :],
            "wproj": np.ascontiguousarray(Wproj.astype(np.float16)),
            "bproj": np.ascontiguousarray(bproj.astype(np.float16))[None, :],
            "ctab": ctab,
            "stab": stab,
            "perm": pmat,
            "tri": tri,
            "ones": np.ones((1, 512), np.float16),
        })
    return maps


def kernel(x, Wqkv, bqkv, Wproj, bproj):
    nc = _get_nc()
    in_maps = _host_prep(x, Wqkv, bqkv, Wproj, bproj)
    res = run_bass_kernel_spmd(nc, in_maps, list(range(8)))
    out = np.empty((B, T, E), np.float32)
    for b in range(B):
        out[b, :T // 2] = res.results[2 * b]["out"][:T // 2]
        out[b, T // 2:] = res.results[2 * b + 1]["out"][T // 2:]
    return out


if __name__ == "__main__":
    rng = np.random.default_rng(0)
    x = rng.standard_normal((B, T, E), dtype=np.float32)
    Wqkv = rng.standard_normal((E, 3 * E), dtype=np.float32) * 0.02
    bqkv = rng.standard_normal((3 * E,), dtype=np.float32) * 0.02
    Wproj = rng.standard_normal((E, E), dtype=np.float32) * 0.02
    bproj = rng.standard_normal((E,), dtype=np.float32) * 0.02
    o = kernel(x=x, Wqkv=Wqkv, bqkv=bqkv, Wproj=Wproj, bproj=bproj)
    print("out", o.shape, o.dtype, float(np.abs(o).max()))


# revision 10
# speedup vs baseline: 1.6004x; 1.1109x over previous
"""Causal self-attention (B=4, T=2048, E=1024, H=16, rope) on 8 trn2 NeuronCores.

Sharding: core c = 2*b + g handles batch b = c//2, head-group g = c%2
(8 of the 16 heads).  Each core:
  - projects its batch's x into q,k (feature-major, rope'd on chip) and v
    for its 8 heads (fp16 matmuls, fp32 accumulate),
  - runs causal attention entirely on-chip (S^T tiles as stationary
    operands, ones-augmented v gives softmax denominators for free),
  - AllGathers the fp16 attention output within the (g=0,g=1) pair and
    applies the full (row-complete) output projection, so no post-proj
    reduce is needed.
Host assembles out[b] from the pair's identical projected outputs.
"""
import sys

for _p in ("/opt/trn_rl_repo", "/root/.axon_site/_ro/trn_rl_repo"):
    if _p not in sys.path:
        sys.path.append(_p)

import numpy as np
from contextlib import ExitStack

import concourse.bass as bass
import concourse.tile as tile
from concourse import bacc, mybir
from concourse.bass_utils import run_bass_kernel_spmd

B, T, E = 4, 2048, 1024
H_TOT, D = 16, 64
HL = 8            # heads per core
F = HL * D        # 512 local q/k/v features
KB = E // 128     # 8 contraction blocks for qkv
TC = T // 512     # 4 time chunks (512 cols)
TT = T // 128     # 16 time tiles
ROPE_THETA = 10000.0

f32 = mybir.dt.float32
f16 = mybir.dt.float16


def build_nc():
    nc = bacc.Bacc(None, target_bir_lowering=False, debug=False)

    xT = nc.declare_dram_parameter("xT", [E, T], f16, isOutput=False)
    wq = nc.declare_dram_parameter("wq", [E, F], f16, isOutput=False)
    wk = nc.declare_dram_parameter("wk", [E, F], f16, isOutput=False)
    wv = nc.declare_dram_parameter("wv", [E, F], f16, isOutput=False)
    bq = nc.declare_dram_parameter("bq", [128, 4], f32, isOutput=False)  # feature-major cols
    bk = nc.declare_dram_parameter("bk", [128, 4], f32, isOutput=False)
    bv = nc.declare_dram_parameter("bv", [1, F], f16, isOutput=False)
    wproj = nc.declare_dram_parameter("wproj", [E, E], f16, isOutput=False)
    bproj = nc.declare_dram_parameter("bproj", [1, E], f16, isOutput=False)
    ctab_d = nc.declare_dram_parameter("ctab", [128, T], f16, isOutput=False)
    stab_d = nc.declare_dram_parameter("stab", [128, T], f16, isOutput=False)
    perm_d = nc.declare_dram_parameter("perm", [128, 128], f16, isOutput=False)
    tri_d = nc.declare_dram_parameter("tri", [128, 128], f16, isOutput=False)  # 0/1 mult mask
    ones_d = nc.declare_dram_parameter("ones", [1, 512], f16, isOutput=False)
    out_ext = nc.declare_dram_parameter("out", [T, E], f32, isOutput=True)

    ag_in = nc.dram_tensor("ag_in", [TC, F, 512], f16)
    ag_out = nc.dram_tensor("ag_out", [TC, 2, F, 512], f16)

    with ExitStack() as ctx:
        tc = ctx.enter_context(tile.TileContext(nc))
        sres = ctx.enter_context(tc.tile_pool(name="res", bufs=1))
        swts = ctx.enter_context(tc.tile_pool(name="wts", bufs=3))
        stab = ctx.enter_context(tc.tile_pool(name="tab", bufs=2))
        sx = ctx.enter_context(tc.tile_pool(name="x", bufs=2))
        stmp = ctx.enter_context(tc.tile_pool(name="tmp", bufs=2))
        sp = ctx.enter_context(tc.tile_pool(name="p", bufs=3))
        sof = ctx.enter_context(tc.tile_pool(name="of", bufs=2))
        sout = ctx.enter_context(tc.tile_pool(name="out", bufs=2))
        ssm = ctx.enter_context(tc.tile_pool(name="sm", bufs=2))
        pps = ctx.enter_context(tc.tile_pool(name="ps", bufs=3, space="PSUM"))
        pac = ctx.enter_context(tc.tile_pool(name="ac", bufs=2, space="PSUM"))

        # ---- resident tiles
        qT_t = sres.tile([128, 4, T], f16, tag="qT")       # rope'd q, feature-major
        kT_t = sres.tile([128, 4, T], f16, tag="kT")
        v_t = sres.tile([128, TT, HL, D + 1], f16, tag="v")  # natural v + ones col
        ot_t = sres.tile([128, 4, T], f16, tag="ot")       # attention out, feature-major

        ones16 = sres.tile([1, 512], f16, tag="ones16")
        nc.gpsimd.dma_start(out=ones16, in_=ones_d[:, :])
        nc.vector.memset(v_t[:, :, :, D:D + 1], 1.0)

        perm_t = sres.tile([128, 128], f16, tag="perm")
        tri_t = sres.tile([128, 128], f16, tag="tri")
        nc.gpsimd.dma_start(out=perm_t, in_=perm_d[:, :])
        nc.gpsimd.dma_start(out=tri_t, in_=tri_d[:, :])

        bq_t = sres.tile([128, 4], f32, tag="bq")
        bk_t = sres.tile([128, 4], f32, tag="bk")
        bv_t = sres.tile([1, F], f16, tag="bv")
        bp_t = sres.tile([1, E], f16, tag="bp")
        nc.gpsimd.dma_start(out=bq_t, in_=bq[:, :])
        nc.gpsimd.dma_start(out=bk_t, in_=bk[:, :])
        nc.gpsimd.dma_start(out=bv_t, in_=bv[:, :])
        nc.gpsimd.dma_start(out=bp_t, in_=bproj[:, :])

        ctab_t = stab.tile([128, T], f16, tag="tab")
        stab_t = stab.tile([128, T], f16, tag="tab")
        nc.gpsimd.dma_start(out=ctab_t, in_=ctab_d[:, :])
        nc.gpsimd.dma_start(out=stab_t, in_=stab_d[:, :])

        wq_t = swts.tile([128, KB, F], f16, tag="w3")
        wk_t = swts.tile([128, KB, F], f16, tag="w3")
        wv_t = swts.tile([128, KB, F], f16, tag="w3")
        xT_r = xT.rearrange("(a p) t -> p a t", p=128)
        wq_r = wq.rearrange("(a p) f -> p a f", p=128)
        wk_r = wk.rearrange("(a p) f -> p a f", p=128)
        wv_r = wv.rearrange("(a p) f -> p a f", p=128)
        for kb in range(KB):
            nc.sync.dma_start(out=wq_t[:, kb, :], in_=wq_r[:, kb, :])
            nc.sync.dma_start(out=wk_t[:, kb, :], in_=wk_r[:, kb, :])
            nc.sync.dma_start(out=wv_t[:, kb, :], in_=wv_r[:, kb, :])

        # ============ interleaved qkv-chunk + attention group per qc ======
        # attention chains (h, qc) only need q/k/v for time < (qc+1)*512,
        # so producing chunk qc then running all 8 heads' chains for qc keeps
        # the PE dense (qkv matmuls fill the gaps the exp pipeline leaves).
        def qkv_chunk(tcx):
            x_t = sx.tile([128, KB, 512], f16, tag="x")
            for kb in range(KB):
                nc.sync.dma_start(out=x_t[:, kb, :],
                                  in_=xT_r[:, kb, tcx * 512:(tcx + 1) * 512])
            cs = slice(tcx * 512, (tcx + 1) * 512)

            for w_t, b_t, dst in ((wq_t, bq_t, qT_t), (wk_t, bk_t, kT_t)):
                for f in range(4):
                    ps2 = pps.tile([128, 1024], f32, tag="mm")
                    ps_q = ps2[:, 0:512]
                    ps_p = ps2[:, 512:1024]
                    for kb in range(KB):
                        nc.tensor.matmul(
                            ps_q,
                            w_t[:, kb, f * 128:(f + 1) * 128],
                            x_t[:, kb, :],
                            start=(kb == 0), stop=(kb == KB - 1),
                        )
                    # bias add (per-partition col) fused into psum->sbuf move
                    q16 = stmp.tile([128, 512], f16, tag="t0")
                    nc.vector.tensor_scalar_add(q16[:, :], ps_q, b_t[:, f:f + 1])
                    nc.tensor.matmul(ps_p, perm_t[:, :], q16[:, :],
                                     start=True, stop=True)
                    t1 = stmp.tile([128, 512], f16, tag="t1")
                    nc.vector.tensor_mul(t1[:, :], q16[:, :], ctab_t[:, cs])
                    t2 = stmp.tile([128, 512], f16, tag="t2")
                    nc.vector.tensor_mul(t2[:, :], ps_p, stab_t[:, cs])
                    nc.vector.tensor_add(dst[:, f, cs], t1[:, :], t2[:, :])

            for tl in range(4):
                tt = tcx * 4 + tl
                ps2 = pps.tile([128, 1024], f32, tag="mm")
                ps_v = ps2[:, 0:512]
                for kb in range(KB):
                    nc.tensor.matmul(
                        ps_v,
                        x_t[:, kb, tl * 128:(tl + 1) * 128],
                        wv_t[:, kb, :],
                        start=(kb == 0), stop=False,
                    )
                nc.tensor.matmul(ps_v, ones16[0:1, 0:128], bv_t[0:1, :],
                                 start=False, stop=True)
                nc.scalar.copy(
                    v_t[:, tt, :, 0:D],
                    ps_v.rearrange("p (h d) -> p h d", h=HL),
                )

        wp_t = swts.tile([128, KB, E], f16, tag="wp")
        wp_r = wproj.rearrange("(a p) e -> p a e", p=128)
        for kb in range(KB):
            nc.sync.dma_start(out=wp_t[:, kb, :], in_=wp_r[:, kb, :])

        def ag_chunk(qc):
            nc.sync.dma_start(
                out=ag_in[qc].rearrange("(a p) t -> p a t", p=128),
                in_=ot_t[:, :, qc * 512:(qc + 1) * 512],
            )
            nc.gpsimd.collective_compute(
                "AllGather",
                mybir.AluOpType.bypass,
                ins=[ag_in[qc]],
                outs=[ag_out[qc]],
                replica_groups=[[0, 1], [2, 3], [4, 5], [6, 7]],
            )

        def proj_chunk(qc):
            ag_r = ag_out[qc].rearrange("s (a p) t -> p (s a) t", p=128)
            for tl in range(4):
                tt = qc * 4 + tl
                of_t = sof.tile([128, KB, 128], f16, tag="of")
                nc.sync.dma_start(out=of_t, in_=ag_r[:, :, tl * 128:(tl + 1) * 128])
                o_st = sout.tile([128, E], f32, tag="o")
                ps2 = pps.tile([128, 1024], f32, tag="mm")
                for nh in range(2):
                    ps_pj = ps2[:, nh * 512:(nh + 1) * 512]
                    ns = slice(nh * 512, (nh + 1) * 512)
                    for kb in range(KB):
                        nc.tensor.matmul(
                            ps_pj,
                            of_t[:, kb, :],
                            wp_t[:, kb, ns],
                            start=(kb == 0), stop=False,
                        )
                    nc.tensor.matmul(ps_pj, ones16[0:1, 0:128], bp_t[0:1, ns],
                                     start=False, stop=True)
                nc.scalar.copy(o_st[:, :], ps2[:, :])
                nc.sync.dma_start(out=out_ext[tt * 128:(tt + 1) * 128, :], in_=o_st)

        for qc in range(TC):
            qkv_chunk(qc)
            qs = slice(qc * 512, (qc + 1) * 512)
            for h in range(HL):
                bp = (h % 2) * 64
                fi = h // 2
                nkt = 4 * qc + 4
                ps_o = pac.tile([D + 1, 512], f32, tag="acc")
                for m in range(nkt // 2):
                    ps_s = pps.tile([128, 1024], f32, tag="mm")
                    p_t = sp.tile([128, 1024], f16, tag="p")
                    for half in range(2):
                        kt = 2 * m + half
                        j = kt - 4 * qc
                        w0 = max(j, 0) * 128
                        nc.tensor.matmul(
                            ps_s[:, half * 512 + w0:half * 512 + 512],
                            kT_t[bp:bp + 64, fi, kt * 128:(kt + 1) * 128],
                            qT_t[bp:bp + 64, fi, qc * 512 + w0:(qc + 1) * 512],
                            start=True, stop=True,
                        )
                    nc.scalar.activation(
                        p_t[:, :], ps_s[:, :],
                        mybir.ActivationFunctionType.Exp, scale=float(D) ** -0.5,
                    )
                    for half in range(2):
                        kt = 2 * m + half
                        j = kt - 4 * qc
                        w0 = max(j, 0) * 128
                        if j >= 0:
                            ms = slice(half * 512 + w0, half * 512 + w0 + 128)
                            nc.vector.tensor_mul(p_t[:, ms], p_t[:, ms], tri_t[:, :])
                        nc.tensor.matmul(
                            ps_o[:, w0:512],
                            v_t[:, kt, h, :],
                            p_t[:, half * 512 + w0:half * 512 + 512],
                            start=(kt == 0), stop=(kt == nkt - 1),
                        )
                # normalize: denominators -> sbuf -> fast reciprocal -> bcast
                sums_sb = ssm.tile([1, 512], f32, tag="sums")
                nc.scalar.copy(sums_sb[:, :], ps_o[D:D + 1, :])
                recip = ssm.tile([1, 512], f32, tag="rc")
                nc.vector.reciprocal_approx_fast(out=recip[:, :], in_=sums_sb[:, :])
                bc = ssm.tile([64, 512], f32, tag="bc")
                nc.gpsimd.partition_broadcast(bc[:, :], recip[:, :])
                nc.vector.tensor_mul(ot_t[bp:bp + 64, fi, qs], ps_o[0:D, :], bc[:, :])
            ag_chunk(qc)
            if qc >= 1:
                proj_chunk(qc - 1)
        proj_chunk(TC - 1)


    nc.compile()
    return nc


_NC = None


def _get_nc():
    global _NC
    if _NC is None:
        _NC = build_nc()
    return _NC


def _host_prep(x, Wqkv, bqkv, Wproj, bproj):
    """Build the 8 per-core input maps."""
    x = np.asarray(x, np.float32)
    Wqkv = np.asarray(Wqkv, np.float32)
    bqkv = np.asarray(bqkv, np.float32)
    Wproj = np.asarray(Wproj, np.float32)
    bproj = np.asarray(bproj, np.float32)

    perm_d = np.concatenate([np.arange(0, D, 2), np.arange(1, D, 2)])  # evens, odds

    # rope tables (feature-major; rows r: freq r%32, sign -/+ per 32-block)
    inv_freq = 1.0 / ROPE_THETA ** (np.arange(0, D, 2, dtype=np.float32) / D)
    freqs = np.arange(T, dtype=np.float32)[:, None] * inv_freq[None, :]  # (T, 32)
    cosf = np.cos(freqs).T.astype(np.float32)  # (32, T)
    sinf = np.sin(freqs).T.astype(np.float32)
    ctab = np.tile(cosf, (4, 1)).astype(np.float16)                 # (128, T)
    stab = np.concatenate([-sinf, sinf, -sinf, sinf], 0).astype(np.float16)

    # block-swap permutation matrix: out row m <- in row pi(m)
    pmat = np.zeros((128, 128), np.float16)
    for m in range(128):
        base = (m // 64) * 64
        r = m % 64
        pmat[base + (r + 32) % 64, m] = 1.0

    tri = (np.arange(128)[:, None] <= np.arange(128)[None, :]).astype(np.float16)

    maps = []
    for c in range(8):
        b, g = c // 2, c % 2
        heads = np.arange(8 * g, 8 * g + 8)
        # permuted q/k columns, natural v columns
        qcols = (heads[:, None] * D + perm_d[None, :]).ravel()
        vcols = (heads[:, None] * D + np.arange(D)[None, :]).ravel()
        maps.append({
            "xT": np.ascontiguousarray(x[b].T.astype(np.float16)),
            "wq": np.ascontiguousarray(Wqkv[:, qcols].astype(np.float16)),
            "wk": np.ascontiguousarray(Wqkv[:, E + qcols].astype(np.float16)),
            "wv": np.ascontiguousarray(Wqkv[:, 2 * E + vcols].astype(np.float16)),
            "bq": np.ascontiguousarray(bqkv[qcols].reshape(4, 128).T.astype(np.float32)),
            "bk": np.ascontiguousarray(bqkv[E + qcols].reshape(4, 128).T.astype(np.float32)),
            "bv": np.ascontiguousarray(bqkv[2 * E + vcols].astype(np.float16))[None, :],
            "wproj": np.ascontiguousarray(Wproj.astype(np.float16)),
            "bproj": np.ascontiguousarray(bproj.astype(np.float16))[None, :],
            "ctab": ctab,
            "stab": stab,
            "perm": pmat,
            "tri": tri,
            "ones": np.ones((1, 512), np.float16),
        })
    return maps


def kernel(x, Wqkv, bqkv, Wproj, bproj):
    nc = _get_nc()
    in_maps = _host_prep(x, Wqkv, bqkv, Wproj, bproj)
    res = run_bass_kernel_spmd(nc, in_maps, list(range(8)))
    out = np.empty((B, T, E), np.float32)
    for b in range(B):
        out[b, :T // 2] = res.results[2 * b]["out"][:T // 2]
        out[b, T // 2:] = res.results[2 * b + 1]["out"][T // 2:]
    return out


if __name__ == "__main__":
    rng = np.random.default_rng(0)
    x = rng.standard_normal((B, T, E), dtype=np.float32)
    Wqkv = rng.standard_normal((E, 3 * E), dtype=np.float32) * 0.02
    bqkv = rng.standard_normal((3 * E,), dtype=np.float32) * 0.02
    Wproj = rng.standard_normal((E, E), dtype=np.float32) * 0.02
    bproj = rng.standard_normal((E,), dtype=np.float32) * 0.02
    o = kernel(x=x, Wqkv=Wqkv, bqkv=bqkv, Wproj=Wproj, bproj=bproj)
    print("out", o.shape, o.dtype, float(np.abs(o).max()))


# revision 11
# speedup vs baseline: 1.6071x; 1.0042x over previous
"""Causal self-attention (B=4, T=2048, E=1024, H=16, rope) on 8 trn2 NeuronCores.

Sharding: core c = 2*b + g handles batch b = c//2, head-group g = c%2
(8 of the 16 heads).  Each core:
  - projects its batch's x into q,k (feature-major, rope'd on chip) and v
    for its 8 heads (fp16 matmuls, fp32 accumulate),
  - runs causal attention entirely on-chip (S^T tiles as stationary
    operands, ones-augmented v gives softmax denominators for free),
  - AllGathers the fp16 attention output within the (g=0,g=1) pair and
    applies the full (row-complete) output projection, so no post-proj
    reduce is needed.
Host assembles out[b] from the pair's identical projected outputs.
"""
import sys

for _p in ("/opt/trn_rl_repo", "/root/.axon_site/_ro/trn_rl_repo"):
    if _p not in sys.path:
        sys.path.append(_p)

import numpy as np
from contextlib import ExitStack

import concourse.bass as bass
import concourse.tile as tile
from concourse import bacc, mybir
from concourse.bass_utils import run_bass_kernel_spmd

B, T, E = 4, 2048, 1024
H_TOT, D = 16, 64
HL = 8            # heads per core
F = HL * D        # 512 local q/k/v features
KB = E // 128     # 8 contraction blocks for qkv
TC = T // 512     # 4 time chunks (512 cols)
TT = T // 128     # 16 time tiles
ROPE_THETA = 10000.0

f32 = mybir.dt.float32
f16 = mybir.dt.float16


def build_nc():
    nc = bacc.Bacc(None, target_bir_lowering=False, debug=False)

    xT = nc.declare_dram_parameter("xT", [E, T], f16, isOutput=False)
    wq = nc.declare_dram_parameter("wq", [E, F], f16, isOutput=False)
    wk = nc.declare_dram_parameter("wk", [E, F], f16, isOutput=False)
    wv = nc.declare_dram_parameter("wv", [E, F], f16, isOutput=False)
    bq = nc.declare_dram_parameter("bq", [128, 4], f32, isOutput=False)  # feature-major cols
    bk = nc.declare_dram_parameter("bk", [128, 4], f32, isOutput=False)
    bv = nc.declare_dram_parameter("bv", [1, F], f16, isOutput=False)
    wproj = nc.declare_dram_parameter("wproj", [E, E], f16, isOutput=False)
    bproj = nc.declare_dram_parameter("bproj", [1, E], f16, isOutput=False)
    ctab_d = nc.declare_dram_parameter("ctab", [128, T], f16, isOutput=False)
    stab_d = nc.declare_dram_parameter("stab", [128, T], f16, isOutput=False)
    perm_d = nc.declare_dram_parameter("perm", [128, 128], f16, isOutput=False)
    tri_d = nc.declare_dram_parameter("tri", [128, 128], f16, isOutput=False)  # 0/1 mult mask
    ones_d = nc.declare_dram_parameter("ones", [1, 512], f16, isOutput=False)
    out_ext = nc.declare_dram_parameter("out", [T, E], f32, isOutput=True)

    ag_in = nc.dram_tensor("ag_in", [TC, F, 512], f16)
    ag_out = nc.dram_tensor("ag_out", [TC, 2, F, 512], f16)

    with ExitStack() as ctx:
        tc = ctx.enter_context(tile.TileContext(nc))
        sres = ctx.enter_context(tc.tile_pool(name="res", bufs=1))
        swts = ctx.enter_context(tc.tile_pool(name="wts", bufs=3))
        stab = ctx.enter_context(tc.tile_pool(name="tab", bufs=2))
        sx = ctx.enter_context(tc.tile_pool(name="x", bufs=2))
        stmp = ctx.enter_context(tc.tile_pool(name="tmp", bufs=2))
        sp = ctx.enter_context(tc.tile_pool(name="p", bufs=3))
        sof = ctx.enter_context(tc.tile_pool(name="of", bufs=2))
        sout = ctx.enter_context(tc.tile_pool(name="out", bufs=2))
        ssm = ctx.enter_context(tc.tile_pool(name="sm", bufs=2))
        pps = ctx.enter_context(tc.tile_pool(name="ps", bufs=2, space="PSUM"))
        pac = ctx.enter_context(tc.tile_pool(name="ac", bufs=4, space="PSUM"))

        # ---- resident tiles
        qT_t = sres.tile([128, 4, T], f16, tag="qT")       # rope'd q, feature-major
        kT_t = sres.tile([128, 4, T], f16, tag="kT")
        v_t = sres.tile([128, TT, HL, D + 1], f16, tag="v")  # natural v + ones col
        ot_t = sres.tile([128, 4, T], f16, tag="ot")       # attention out, feature-major

        ones16 = sres.tile([1, 512], f16, tag="ones16")
        nc.gpsimd.dma_start(out=ones16, in_=ones_d[:, :])
        nc.vector.memset(v_t[:, :, :, D:D + 1], 1.0)

        perm_t = sres.tile([128, 128], f16, tag="perm")
        tri_t = sres.tile([128, 128], f16, tag="tri")
        nc.gpsimd.dma_start(out=perm_t, in_=perm_d[:, :])
        nc.gpsimd.dma_start(out=tri_t, in_=tri_d[:, :])

        bq_t = sres.tile([128, 4], f32, tag="bq")
        bk_t = sres.tile([128, 4], f32, tag="bk")
        bv_t = sres.tile([1, F], f16, tag="bv")
        bp_t = sres.tile([1, E], f16, tag="bp")
        nc.gpsimd.dma_start(out=bq_t, in_=bq[:, :])
        nc.gpsimd.dma_start(out=bk_t, in_=bk[:, :])
        nc.gpsimd.dma_start(out=bv_t, in_=bv[:, :])
        nc.gpsimd.dma_start(out=bp_t, in_=bproj[:, :])

        ctab_t = stab.tile([128, T], f16, tag="tab")
        stab_t = stab.tile([128, T], f16, tag="tab")
        nc.gpsimd.dma_start(out=ctab_t, in_=ctab_d[:, :])
        nc.gpsimd.dma_start(out=stab_t, in_=stab_d[:, :])

        wq_t = swts.tile([128, KB, F], f16, tag="w3")
        wk_t = swts.tile([128, KB, F], f16, tag="w3")
        wv_t = swts.tile([128, KB, F], f16, tag="w3")
        xT_r = xT.rearrange("(a p) t -> p a t", p=128)
        wq_r = wq.rearrange("(a p) f -> p a f", p=128)
        wk_r = wk.rearrange("(a p) f -> p a f", p=128)
        wv_r = wv.rearrange("(a p) f -> p a f", p=128)
        for kb in range(KB):
            nc.sync.dma_start(out=wq_t[:, kb, :], in_=wq_r[:, kb, :])
            nc.sync.dma_start(out=wk_t[:, kb, :], in_=wk_r[:, kb, :])
            nc.sync.dma_start(out=wv_t[:, kb, :], in_=wv_r[:, kb, :])

        # ============ interleaved qkv-chunk + attention group per qc ======
        # attention chains (h, qc) only need q/k/v for time < (qc+1)*512,
        # so producing chunk qc then running all 8 heads' chains for qc keeps
        # the PE dense (qkv matmuls fill the gaps the exp pipeline leaves).
        def qkv_chunk(tcx):
            x_t = sx.tile([128, KB, 512], f16, tag="x")
            for kb in range(KB):
                nc.sync.dma_start(out=x_t[:, kb, :],
                                  in_=xT_r[:, kb, tcx * 512:(tcx + 1) * 512])
            cs = slice(tcx * 512, (tcx + 1) * 512)

            for w_t, b_t, dst in ((wq_t, bq_t, qT_t), (wk_t, bk_t, kT_t)):
                for f in range(4):
                    ps2 = pps.tile([128, 1024], f32, tag="mm")
                    ps_q = ps2[:, 0:512]
                    ps_p = ps2[:, 512:1024]
                    for kb in range(KB):
                        nc.tensor.matmul(
                            ps_q,
                            w_t[:, kb, f * 128:(f + 1) * 128],
                            x_t[:, kb, :],
                            start=(kb == 0), stop=(kb == KB - 1),
                        )
                    # bias add (per-partition col) fused into psum->sbuf move
                    q16 = stmp.tile([128, 512], f16, tag="t0")
                    nc.vector.tensor_scalar_add(q16[:, :], ps_q, b_t[:, f:f + 1])
                    nc.tensor.matmul(ps_p, perm_t[:, :], q16[:, :],
                                     start=True, stop=True)
                    t1 = stmp.tile([128, 512], f16, tag="t1")
                    nc.vector.tensor_mul(t1[:, :], q16[:, :], ctab_t[:, cs])
                    t2 = stmp.tile([128, 512], f16, tag="t2")
                    nc.vector.tensor_mul(t2[:, :], ps_p, stab_t[:, cs])
                    nc.vector.tensor_add(dst[:, f, cs], t1[:, :], t2[:, :])

            for tl in range(4):
                tt = tcx * 4 + tl
                ps2 = pps.tile([128, 1024], f32, tag="mm")
                ps_v = ps2[:, 0:512]
                for kb in range(KB):
                    nc.tensor.matmul(
                        ps_v,
                        x_t[:, kb, tl * 128:(tl + 1) * 128],
                        wv_t[:, kb, :],
                        start=(kb == 0), stop=False,
                    )
                nc.tensor.matmul(ps_v, ones16[0:1, 0:128], bv_t[0:1, :],
                                 start=False, stop=True)
                nc.scalar.copy(
                    v_t[:, tt, :, 0:D],
                    ps_v.rearrange("p (h d) -> p h d", h=HL),
                )

        wp_t = swts.tile([128, KB, E], f16, tag="wp")
        wp_r = wproj.rearrange("(a p) e -> p a e", p=128)

        def load_wp():
            for kb in range(KB):
                nc.sync.dma_start(out=wp_t[:, kb, :], in_=wp_r[:, kb, :])

        def ag_chunk(qc):
            nc.sync.dma_start(
                out=ag_in[qc].rearrange("(a p) t -> p a t", p=128),
                in_=ot_t[:, :, qc * 512:(qc + 1) * 512],
            )
            nc.gpsimd.collective_compute(
                "AllGather",
                mybir.AluOpType.bypass,
                ins=[ag_in[qc]],
                outs=[ag_out[qc]],
                replica_groups=[[0, 1], [2, 3], [4, 5], [6, 7]],
            )

        def proj_chunk(qc):
            ag_r = ag_out[qc].rearrange("s (a p) t -> p (s a) t", p=128)
            for tl in range(4):
                tt = qc * 4 + tl
                of_t = sof.tile([128, KB, 128], f16, tag="of")
                nc.sync.dma_start(out=of_t, in_=ag_r[:, :, tl * 128:(tl + 1) * 128])
                o_st = sout.tile([128, E], f32, tag="o")
                ps2 = pps.tile([128, 1024], f32, tag="mm")
                for nh in range(2):
                    ps_pj = ps2[:, nh * 512:(nh + 1) * 512]
                    ns = slice(nh * 512, (nh + 1) * 512)
                    for kb in range(KB):
                        nc.tensor.matmul(
                            ps_pj,
                            of_t[:, kb, :],
                            wp_t[:, kb, ns],
                            start=(kb == 0), stop=False,
                        )
                    nc.tensor.matmul(ps_pj, ones16[0:1, 0:128], bp_t[0:1, ns],
                                     start=False, stop=True)
                nc.scalar.copy(o_st[:, :], ps2[:, :])
                nc.sync.dma_start(out=out_ext[tt * 128:(tt + 1) * 128, :], in_=o_st)

        for qc in range(TC):
            qkv_chunk(qc)
            qs = slice(qc * 512, (qc + 1) * 512)
            for hp in range(HL // 2):
                fi = hp
                hA, hB = 2 * hp, 2 * hp + 1
                nkt = 4 * qc + 4
                ps_oA = pac.tile([D + 1, 512], f32, tag="acc")
                ps_oB = pac.tile([D + 1, 512], f32, tag="acc")
                for kt in range(nkt):
                    j = kt - 4 * qc
                    w0 = max(j, 0) * 128
                    ps_s = pps.tile([128, 1024], f32, tag="mm")
                    p_t = sp.tile([128, 1024], f16, tag="p")
                    # both heads' S^T tiles run concurrently (row strips 0/64)
                    for half, bp in ((0, 0), (1, 64)):
                        nc.tensor.matmul(
                            ps_s[:, half * 512 + w0:half * 512 + 512],
                            kT_t[bp:bp + 64, fi, kt * 128:(kt + 1) * 128],
                            qT_t[bp:bp + 64, fi, qc * 512 + w0:(qc + 1) * 512],
                            start=True, stop=True,
                        )
                    nc.scalar.activation(
                        p_t[:, :], ps_s[:, :],
                        mybir.ActivationFunctionType.Exp, scale=float(D) ** -0.5,
                    )
                    if j >= 0:
                        for half in range(2):
                            ms = slice(half * 512 + w0, half * 512 + w0 + 128)
                            nc.vector.tensor_mul(p_t[:, ms], p_t[:, ms], tri_t[:, :])
                    for half, h, ps_o in ((0, hA, ps_oA), (1, hB, ps_oB)):
                        nc.tensor.matmul(
                            ps_o[:, w0:512],
                            v_t[:, kt, h, :],
                            p_t[:, half * 512 + w0:half * 512 + 512],
                            start=(kt == 0), stop=(kt == nkt - 1),
                        )
                # normalize: denominators -> sbuf -> fast reciprocal -> bcast
                for bp, ps_o in ((0, ps_oA), (64, ps_oB)):
                    sums_sb = ssm.tile([1, 512], f32, tag="sums")
                    nc.vector.tensor_copy(sums_sb[:, :], ps_o[D:D + 1, :])
                    recip = ssm.tile([1, 512], f32, tag="rc")
                    nc.vector.reciprocal_approx_fast(out=recip[:, :], in_=sums_sb[:, :])
                    bc = ssm.tile([64, 512], f32, tag="bc")
                    nc.gpsimd.partition_broadcast(bc[:, :], recip[:, :])
                    nc.vector.tensor_mul(ot_t[bp:bp + 64, fi, qs], ps_o[0:D, :], bc[:, :])
            ag_chunk(qc)
            if qc == 0:
                load_wp()
            if qc >= 1:
                proj_chunk(qc - 1)
        proj_chunk(TC - 1)


    nc.compile()
    return nc


_NC = None


def _get_nc():
    global _NC
    if _NC is None:
        _NC = build_nc()
    return _NC


def _host_prep(x, Wqkv, bqkv, Wproj, bproj):
    """Build the 8 per-core input maps."""
    x = np.asarray(x, np.float32)
    Wqkv = np.asarray(Wqkv, np.float32)
    bqkv = np.asarray(bqkv, np.float32)
    Wproj = np.asarray(Wproj, np.float32)
    bproj = np.asarray(bproj, np.float32)

    perm_d = np.concatenate([np.arange(0, D, 2), np.arange(1, D, 2)])  # evens, odds

    # rope tables (feature-major; rows r: freq r%32, sign -/+ per 32-block)
    inv_freq = 1.0 / ROPE_THETA ** (np.arange(0, D, 2, dtype=np.float32) / D)
    freqs = np.arange(T, dtype=np.float32)[:, None] * inv_freq[None, :]  # (T, 32)
    cosf = np.cos(freqs).T.astype(np.float32)  # (32, T)
    sinf = np.sin(freqs).T.astype(np.float32)
    ctab = np.tile(cosf, (4, 1)).astype(np.float16)                 # (128, T)
    stab = np.concatenate([-sinf, sinf, -sinf, sinf], 0).astype(np.float16)

    # block-swap permutation matrix: out row m <- in row pi(m)
    pmat = np.zeros((128, 128), np.float16)
    for m in range(128):
        base = (m // 64) * 64
        r = m % 64
        pmat[base + (r + 32) % 64, m] = 1.0

    tri = (np.arange(128)[:, None] <= np.arange(128)[None, :]).astype(np.float16)

    maps = []
    for c in range(8):
        b, g = c // 2, c % 2
        heads = np.arange(8 * g, 8 * g + 8)
        # permuted q/k columns, natural v columns
        qcols = (heads[:, None] * D + perm_d[None, :]).ravel()
        vcols = (heads[:, None] * D + np.arange(D)[None, :]).ravel()
        maps.append({
            "xT": np.ascontiguousarray(x[b].T.astype(np.float16)),
            "wq": np.ascontiguousarray(Wqkv[:, qcols].astype(np.float16)),
            "wk": np.ascontiguousarray(Wqkv[:, E + qcols].astype(np.float16)),
            "wv": np.ascontiguousarray(Wqkv[:, 2 * E + vcols].astype(np.float16)),
            "bq": np.ascontiguousarray(bqkv[qcols].reshape(4, 128).T.astype(np.float32)),
            "bk": np.ascontiguousarray(bqkv[E + qcols].reshape(4, 128).T.astype(np.float32)),
            "bv": np.ascontiguousarray(bqkv[2 * E + vcols].astype(np.float16))[None, :],
            "wproj": np.ascontiguousarray(Wproj.astype(np.float16)),
            "bproj": np.ascontiguousarray(bproj.astype(np.float16))[None, :],
            "ctab": ctab,
            "stab": stab,
            "perm": pmat,
            "tri": tri,
            "ones": np.ones((1, 512), np.float16),
        })
    return maps


def kernel(x, Wqkv, bqkv, Wproj, bproj):
    nc = _get_nc()
    in_maps = _host_prep(x, Wqkv, bqkv, Wproj, bproj)
    res = run_bass_kernel_spmd(nc, in_maps, list(range(8)))
    out = np.empty((B, T, E), np.float32)
    for b in range(B):
        out[b, :T // 2] = res.results[2 * b]["out"][:T // 2]
        out[b, T // 2:] = res.results[2 * b + 1]["out"][T // 2:]
    return out


if __name__ == "__main__":
    rng = np.random.default_rng(0)
    x = rng.standard_normal((B, T, E), dtype=np.float32)
    Wqkv = rng.standard_normal((E, 3 * E), dtype=np.float32) * 0.02
    bqkv = rng.standard_normal((3 * E,), dtype=np.float32) * 0.02
    Wproj = rng.standard_normal((E, E), dtype=np.float32) * 0.02
    bproj = rng.standard_normal((E,), dtype=np.float32) * 0.02
    o = kernel(x=x, Wqkv=Wqkv, bqkv=bqkv, Wproj=Wproj, bproj=bproj)
    print("out", o.shape, o.dtype, float(np.abs(o).max()))


# revision 14
# speedup vs baseline: 1.6394x; 1.0201x over previous
"""Causal self-attention (B=4, T=2048, E=1024, H=16, rope) on 8 trn2 NeuronCores.

Sharding: core c = 2*b + g handles batch b = c//2, head-group g = c%2
(8 of the 16 heads).  Each core:
  - projects its batch's x into q,k (feature-major, rope'd on chip) and v
    for its 8 heads (fp16 matmuls, fp32 accumulate),
  - runs causal attention entirely on-chip (S^T tiles as stationary
    operands, ones-augmented v gives softmax denominators for free),
  - AllGathers the fp16 attention output within the (g=0,g=1) pair and
    applies the full (row-complete) output projection, so no post-proj
    reduce is needed.
Host assembles out[b] from the pair's identical projected outputs.
"""
import sys

for _p in ("/opt/trn_rl_repo", "/root/.axon_site/_ro/trn_rl_repo"):
    if _p not in sys.path:
        sys.path.append(_p)

import numpy as np
from contextlib import ExitStack

import concourse.bass as bass
import concourse.tile as tile
from concourse import bacc, mybir
from concourse.bass_utils import run_bass_kernel_spmd

B, T, E = 4, 2048, 1024
H_TOT, D = 16, 64
HL = 8            # heads per core
F = HL * D        # 512 local q/k/v features
KB = E // 128     # 8 contraction blocks for qkv
TC = T // 512     # 4 time chunks (512 cols)
TT = T // 128     # 16 time tiles
ROPE_THETA = 10000.0

f32 = mybir.dt.float32
f16 = mybir.dt.float16


def build_nc():
    nc = bacc.Bacc(None, target_bir_lowering=False, debug=False)

    xT = nc.declare_dram_parameter("xT", [E, T], f16, isOutput=False)
    wq = nc.declare_dram_parameter("wq", [E, F], f16, isOutput=False)
    wk = nc.declare_dram_parameter("wk", [E, F], f16, isOutput=False)
    wv = nc.declare_dram_parameter("wv", [E, F], f16, isOutput=False)
    bq = nc.declare_dram_parameter("bq", [128, 4], f32, isOutput=False)  # feature-major cols
    bk = nc.declare_dram_parameter("bk", [128, 4], f32, isOutput=False)
    bv = nc.declare_dram_parameter("bv", [1, F], f16, isOutput=False)
    wproj = nc.declare_dram_parameter("wproj", [E, E], f16, isOutput=False)
    bproj = nc.declare_dram_parameter("bproj", [1, E], f16, isOutput=False)
    ctab_d = nc.declare_dram_parameter("ctab", [128, T], f16, isOutput=False)
    stab_d = nc.declare_dram_parameter("stab", [128, T], f16, isOutput=False)
    perm_d = nc.declare_dram_parameter("perm", [128, 128], f16, isOutput=False)
    tri_d = nc.declare_dram_parameter("tri", [128, 128], f16, isOutput=False)  # 0/1 mult mask
    ones_d = nc.declare_dram_parameter("ones", [1, 512], f16, isOutput=False)
    out_ext = nc.declare_dram_parameter("out", [T, E], f32, isOutput=True)

    GROUPS = [(0, 512), (512, 512), (1024, 512), (1536, 512)]
    ag_in = [nc.dram_tensor(f"ag_in{i}", [F, w], f16) for i, (q0, w) in enumerate(GROUPS)]
    ag_out = [nc.dram_tensor(f"ag_out{i}", [2, F, w], f16) for i, (q0, w) in enumerate(GROUPS)]

    with ExitStack() as ctx:
        tc = ctx.enter_context(tile.TileContext(nc))
        sres = ctx.enter_context(tc.tile_pool(name="res", bufs=1))
        swts = ctx.enter_context(tc.tile_pool(name="wts", bufs=3))
        stab = ctx.enter_context(tc.tile_pool(name="tab", bufs=2))
        sx = ctx.enter_context(tc.tile_pool(name="x", bufs=2))
        stmp = ctx.enter_context(tc.tile_pool(name="tmp", bufs=2))
        sp = ctx.enter_context(tc.tile_pool(name="p", bufs=3))
        sof = ctx.enter_context(tc.tile_pool(name="of", bufs=2))
        sout = ctx.enter_context(tc.tile_pool(name="out", bufs=2))
        ssm = ctx.enter_context(tc.tile_pool(name="sm", bufs=2))
        pps = ctx.enter_context(tc.tile_pool(name="ps", bufs=2, space="PSUM"))
        pac = ctx.enter_context(tc.tile_pool(name="ac", bufs=4, space="PSUM"))

        # ---- resident tiles
        qT_t = sres.tile([128, 4, T], f16, tag="qT")       # rope'd q, feature-major
        kT_t = sres.tile([128, 4, T], f16, tag="kT")
        v_t = sres.tile([128, TT, HL, D + 1], f16, tag="v")  # natural v + ones col
        ot_t = sres.tile([128, 4, T], f16, tag="ot")       # attention out, feature-major

        ones16 = sres.tile([1, 512], f16, tag="ones16")
        nc.gpsimd.dma_start(out=ones16, in_=ones_d[:, :])
        nc.vector.memset(v_t[:, :, :, D:D + 1], 1.0)

        perm_t = sres.tile([128, 128], f16, tag="perm")
        tri_t = sres.tile([128, 128], f16, tag="tri")
        nc.gpsimd.dma_start(out=perm_t, in_=perm_d[:, :])
        nc.gpsimd.dma_start(out=tri_t, in_=tri_d[:, :])

        bq_t = sres.tile([128, 4], f32, tag="bq")
        bk_t = sres.tile([128, 4], f32, tag="bk")
        bv_t = sres.tile([1, F], f16, tag="bv")
        bp_t = sres.tile([1, E], f16, tag="bp")
        nc.gpsimd.dma_start(out=bq_t, in_=bq[:, :])
        nc.gpsimd.dma_start(out=bk_t, in_=bk[:, :])
        nc.gpsimd.dma_start(out=bv_t, in_=bv[:, :])
        nc.gpsimd.dma_start(out=bp_t, in_=bproj[:, :])

        ctab_t = stab.tile([128, T], f16, tag="tab")
        stab_t = stab.tile([128, T], f16, tag="tab")

        wq_t = swts.tile([128, KB, F], f16, tag="w3")
        wk_t = swts.tile([128, KB, F], f16, tag="w3")
        wv_t = swts.tile([128, KB, F], f16, tag="w3")
        xT_r = xT.rearrange("(a p) t -> p a t", p=128)
        wq_r = wq.rearrange("(a p) f -> p a f", p=128)
        wk_r = wk.rearrange("(a p) f -> p a f", p=128)
        wv_r = wv.rearrange("(a p) f -> p a f", p=128)
        for kb in range(KB):
            nc.sync.dma_start(out=wq_t[:, kb, :], in_=wq_r[:, kb, :])
            nc.sync.dma_start(out=wk_t[:, kb, :], in_=wk_r[:, kb, :])
            nc.sync.dma_start(out=wv_t[:, kb, :], in_=wv_r[:, kb, :])

        # ============ interleaved qkv-chunk + attention group per qc ======
        # attention chains (h, qc) only need q/k/v for time < (qc+1)*512,
        # so producing chunk qc then running all 8 heads' chains for qc keeps
        # the PE dense (qkv matmuls fill the gaps the exp pipeline leaves).
        def qkv_chunk(tcx):
            x_t = sx.tile([128, KB, 512], f16, tag="x")
            cs = slice(tcx * 512, (tcx + 1) * 512)
            nc.sync.dma_start(out=ctab_t[:, cs], in_=ctab_d[:, cs])
            nc.sync.dma_start(out=stab_t[:, cs], in_=stab_d[:, cs])
            for kb in range(KB):
                nc.sync.dma_start(out=x_t[:, kb, :],
                                  in_=xT_r[:, kb, tcx * 512:(tcx + 1) * 512])

            for w_t, b_t, dst in ((wq_t, bq_t, qT_t), (wk_t, bk_t, kT_t)):
                for f in range(4):
                    ps2 = pps.tile([128, 1024], f32, tag="mm")
                    ps_q = ps2[:, 0:512]
                    ps_p = ps2[:, 512:1024]
                    for kb in range(KB):
                        nc.tensor.matmul(
                            ps_q,
                            w_t[:, kb, f * 128:(f + 1) * 128],
                            x_t[:, kb, :],
                            start=(kb == 0), stop=(kb == KB - 1),
                        )
                    # bias add (per-partition col) fused into psum->sbuf move
                    q16 = stmp.tile([128, 512], f16, tag="t0")
                    nc.vector.tensor_scalar_add(q16[:, :], ps_q, b_t[:, f:f + 1])
                    nc.tensor.matmul(ps_p, perm_t[:, :], q16[:, :],
                                     start=True, stop=True)
                    t1 = stmp.tile([128, 512], f16, tag="t1")
                    nc.vector.tensor_mul(t1[:, :], q16[:, :], ctab_t[:, cs])
                    t2 = stmp.tile([128, 512], f16, tag="t2")
                    nc.vector.tensor_mul(t2[:, :], ps_p, stab_t[:, cs])
                    nc.vector.tensor_add(dst[:, f, cs], t1[:, :], t2[:, :])

            for tl in range(4):
                tt = tcx * 4 + tl
                ps2 = pps.tile([128, 1024], f32, tag="mm")
                ps_v = ps2[:, 0:512]
                for kb in range(KB):
                    nc.tensor.matmul(
                        ps_v,
                        x_t[:, kb, tl * 128:(tl + 1) * 128],
                        wv_t[:, kb, :],
                        start=(kb == 0), stop=False,
                    )
                nc.tensor.matmul(ps_v, ones16[0:1, 0:128], bv_t[0:1, :],
                                 start=False, stop=True)
                nc.scalar.copy(
                    v_t[:, tt, :, 0:D],
                    ps_v.rearrange("p (h d) -> p h d", h=HL),
                )

        wp_t = swts.tile([128, KB, E], f16, tag="wp")
        wp_r = wproj.rearrange("(a p) e -> p a e", p=128)

        def load_wp():
            for kb in range(KB):
                nc.sync.dma_start(out=wp_t[:, kb, :], in_=wp_r[:, kb, :])

        def ag_chunk(gi):
            q0, w = GROUPS[gi]
            nc.sync.dma_start(
                out=ag_in[gi].rearrange("(a p) t -> p a t", p=128),
                in_=ot_t[:, :, q0:q0 + w],
            )
            nc.gpsimd.collective_compute(
                "AllGather",
                mybir.AluOpType.bypass,
                ins=[ag_in[gi][:, :]],
                outs=[ag_out[gi][:, :, :]],
                replica_groups=[[0, 1], [2, 3], [4, 5], [6, 7]],
            )

        def proj_chunk(gi):
            q0, w = GROUPS[gi]
            ag_r = ag_out[gi].rearrange("s (a p) t -> p (s a) t", p=128)
            for tl in range(w // 128):
                tt = q0 // 128 + tl
                of_t = sof.tile([128, KB, 128], f16, tag="of")
                nc.sync.dma_start(out=of_t, in_=ag_r[:, :, tl * 128:(tl + 1) * 128])
                o_st = sout.tile([128, E], f32, tag="o")
                ps2 = pps.tile([128, 1024], f32, tag="mm")
                for nh in range(2):
                    ps_pj = ps2[:, nh * 512:(nh + 1) * 512]
                    ns = slice(nh * 512, (nh + 1) * 512)
                    for kb in range(KB):
                        nc.tensor.matmul(
                            ps_pj,
                            of_t[:, kb, :],
                            wp_t[:, kb, ns],
                            start=(kb == 0), stop=False,
                        )
                    nc.tensor.matmul(ps_pj, ones16[0:1, 0:128], bp_t[0:1, ns],
                                     start=False, stop=True)
                nc.scalar.copy(o_st[:, :], ps2[:, :])
                nc.sync.dma_start(out=out_ext[tt * 128:(tt + 1) * 128, :], in_=o_st)

        def attn_group(gi):
            q0, w = GROUPS[gi]
            qs = slice(q0, q0 + w)
            kt0 = q0 // 128
            for hp in range(HL // 2):
                fi = hp
                hA, hB = 2 * hp, 2 * hp + 1
                nkt = kt0 + w // 128
                ps_oA = pac.tile([D + 1, 512], f32, tag="acc")
                ps_oB = pac.tile([D + 1, 512], f32, tag="acc")
                for kt in range(nkt):
                    j = kt - kt0
                    w0 = max(j, 0) * 128
                    ps_s = pps.tile([128, 1024], f32, tag="mm")
                    p_t = sp.tile([128, 1024], f16, tag="p")
                    # both heads' S^T tiles run concurrently (row strips 0/64)
                    for half, bp in ((0, 0), (1, 64)):
                        nc.tensor.matmul(
                            ps_s[:, half * w + w0:half * w + w],
                            kT_t[bp:bp + 64, fi, kt * 128:(kt + 1) * 128],
                            qT_t[bp:bp + 64, fi, q0 + w0:q0 + w],
                            start=True, stop=True,
                        )
                    nc.scalar.activation(
                        p_t[:, 0:2 * w], ps_s[:, 0:2 * w],
                        mybir.ActivationFunctionType.Exp, scale=float(D) ** -0.5,
                    )
                    if j >= 0:
                        for half in range(2):
                            ms = slice(half * w + w0, half * w + w0 + 128)
                            nc.vector.tensor_mul(p_t[:, ms], p_t[:, ms], tri_t[:, :])
                    for half, h, ps_o in ((0, hA, ps_oA), (1, hB, ps_oB)):
                        nc.tensor.matmul(
                            ps_o[:, w0:w],
                            v_t[:, kt, h, :],
                            p_t[:, half * w + w0:half * w + w],
                            start=(kt == 0), stop=(kt == nkt - 1),
                        )
                # normalize: denominators -> sbuf -> fast reciprocal -> bcast
                for bp, ps_o in ((0, ps_oA), (64, ps_oB)):
                    sums_sb = ssm.tile([1, 512], f32, tag="sums")
                    nc.vector.tensor_copy(sums_sb[:, 0:w], ps_o[D:D + 1, 0:w])
                    recip = ssm.tile([1, 512], f32, tag="rc")
                    nc.vector.reciprocal_approx_fast(out=recip[:, 0:w], in_=sums_sb[:, 0:w])
                    bc = ssm.tile([64, 512], f32, tag="bc")
                    nc.gpsimd.partition_broadcast(bc[:, 0:w], recip[:, 0:w])
                    nc.vector.tensor_mul(ot_t[bp:bp + 64, fi, qs], ps_o[0:D, 0:w], bc[:, 0:w])

        CHUNK_GROUPS = {0: [0], 1: [1], 2: [2], 3: [3]}
        done = 0
        for tcx in range(TC):
            qkv_chunk(tcx)
            for gi in CHUNK_GROUPS[tcx]:
                attn_group(gi)
                ag_chunk(gi)
                if gi == 0:
                    load_wp()
                if gi >= 1:
                    proj_chunk(gi - 1)
        proj_chunk(len(GROUPS) - 1)


    nc.compile()
    return nc


_NC = None


def _get_nc():
    global _NC
    if _NC is None:
        _NC = build_nc()
    return _NC


def _host_prep(x, Wqkv, bqkv, Wproj, bproj):
    """Build the 8 per-core input maps."""
    x = np.asarray(x, np.float32)
    Wqkv = np.asarray(Wqkv, np.float32)
    bqkv = np.asarray(bqkv, np.float32)
    Wproj = np.asarray(Wproj, np.float32)
    bproj = np.asarray(bproj, np.float32)

    perm_d = np.concatenate([np.arange(0, D, 2), np.arange(1, D, 2)])  # evens, odds

    # rope tables (feature-major; rows r: freq r%32, sign -/+ per 32-block)
    inv_freq = 1.0 / ROPE_THETA ** (np.arange(0, D, 2, dtype=np.float32) / D)
    freqs = np.arange(T, dtype=np.float32)[:, None] * inv_freq[None, :]  # (T, 32)
    cosf = np.cos(freqs).T.astype(np.float32)  # (32, T)
    sinf = np.sin(freqs).T.astype(np.float32)
    ctab = np.tile(cosf, (4, 1)).astype(np.float16)                 # (128, T)
    stab = np.concatenate([-sinf, sinf, -sinf, sinf], 0).astype(np.float16)

    # block-swap permutation matrix: out row m <- in row pi(m)
    pmat = np.zeros((128, 128), np.float16)
    for m in range(128):
        base = (m // 64) * 64
        r = m % 64
        pmat[base + (r + 32) % 64, m] = 1.0

    tri = (np.arange(128)[:, None] <= np.arange(128)[None, :]).astype(np.float16)

    maps = []
    for c in range(8):
        b, g = c // 2, c % 2
        heads = np.arange(8 * g, 8 * g + 8)
        # permuted q/k columns, natural v columns
        qcols = (heads[:, None] * D + perm_d[None, :]).ravel()
        vcols = (heads[:, None] * D + np.arange(D)[None, :]).ravel()
        maps.append({
            "xT": np.ascontiguousarray(x[b].T.astype(np.float16)),
            "wq": np.ascontiguousarray(Wqkv[:, qcols].astype(np.float16)),
            "wk": np.ascontiguousarray(Wqkv[:, E + qcols].astype(np.float16)),
            "wv": np.ascontiguousarray(Wqkv[:, 2 * E + vcols].astype(np.float16)),
            "bq": np.ascontiguousarray(bqkv[qcols].reshape(4, 128).T.astype(np.float32)),
            "bk": np.ascontiguousarray(bqkv[E + qcols].reshape(4, 128).T.astype(np.float32)),
            "bv": np.ascontiguousarray(bqkv[2 * E + vcols].astype(np.float16))[None, :],
            "wproj": np.ascontiguousarray(Wproj.astype(np.float16)),
            "bproj": np.ascontiguousarray(bproj.astype(np.float16))[None, :],
            "ctab": ctab,
            "stab": stab,
            "perm": pmat,
            "tri": tri,
            "ones": np.ones((1, 512), np.float16),
        })
    return maps


def kernel(x, Wqkv, bqkv, Wproj, bproj):
    nc = _get_nc()
    in_maps = _host_prep(x, Wqkv, bqkv, Wproj, bproj)
    res = run_bass_kernel_spmd(nc, in_maps, list(range(8)))
    out = np.empty((B, T, E), np.float32)
    for b in range(B):
        out[b, :T // 2] = res.results[2 * b]["out"][:T // 2]
        out[b, T // 2:] = res.results[2 * b + 1]["out"][T // 2:]
    return out


if __name__ == "__main__":
    rng = np.random.default_rng(0)
    x = rng.standard_normal((B, T, E), dtype=np.float32)
    Wqkv = rng.standard_normal((E, 3 * E), dtype=np.float32) * 0.02
    bqkv = rng.standard_normal((3 * E,), dtype=np.float32) * 0.02
    Wproj = rng.standard_normal((E, E), dtype=np.float32) * 0.02
    bproj = rng.standard_normal((E,), dtype=np.float32) * 0.02
    o = kernel(x=x, Wqkv=Wqkv, bqkv=bqkv, Wproj=Wproj, bproj=bproj)
    print("out", o.shape, o.dtype, float(np.abs(o).max()))
